# revision 24
# baseline (speedup 1.0000x reference)
"""Bass/Trainium2 kernel for a binarized NN (BNN) forward pass, data-parallel
over 8 NeuronCores.

Reference semantics (fp32):
    h1 = x @ sign(W1).T;  b1 = sign(h1 - mean(h1, axis=0))        # g=1, b=0
    h2 = b1 @ sign(W2).T; b2 = noisy_sign(h2, u2)                  # BN+sign is
    h3 = b2 @ sign(W3).T; b3 = noisy_sign(h3, u3)                  # identity on +-1
    out = b3 @ sign(W4).T

Implementation summary (all arithmetic exact small-integer, as in the
fixed-point analysis below):
  * Layer 1 contracts a balanced base-32 fixed-point split of x:
    round(x*2^21) = sum_j d_j 32^(4-j) with digits d_j in [-16,16], every
    one exact in fp8e4m3.  Two PSUM groups share one stored weight stream
    [32*w, w, w/32] (w = sign(W1)): group A moves digits (d0,d1,d2) over
    the full stream, group B moves (d3,d4) over its 2/3 prefix, so
    h1*2^21 = 2^15*A + B.  All products and DoubleRow pair-sums are
    integers scaled by powers of two within the pair-adder's ~12-bit
    mantissa, and each group's PSUM total stays under 2^24/granularity,
    so accumulation is exact.  17 DR pair-matmuls per output chunk
    (vs 20 for the 6-nibble scheme).
  * ACT stages bsb = 2^15*A + c1 (c1 = -2^21*mean(h1), computed on host in
    float64); a single custom DVE op then emits
    b1 = clip((B + bsb) * 2^35, -1, 1) = sign(h1 - mu1) directly.  The
    only inexactness is one fp32 rounding in the stage/add (~2^-19 of h1)
    plus the 2^-22 input quantization - both far below the reference's
    own matmul rounding scale.
  * W2/W3 ship as +-0.5 so PSUM holds h' = h/2 (an exact integer <= 512).
    The stochastic flip (u < 0.5 exp(-h^2/50), |h| <= 50) depends only on
    A(u) = smallest even a with p(a) <= u: flip <=> |h| < A.  With
    t = h' - 1/4 and T2 ~ ((A-1)/2)^2 (fp8-rounded inside its decision
    margin, -1 when A = 0),
        noisy = clip(32 * t * (t*t - T2), -1, 1)
    equals the exact noisy sign for every integer h, so the u-derived
    tables ship as fp8, halving their HBM traffic vs bf16.
  * b in {+-1} and sign(W4) in {+-1} make the output exact integers.

Layout is feature-major: activations live as [features(partitions),
batch(free)].  Batch 16384 is sharded 2048/core; each core pipelines four
512-column slices through all four layers, skewed one slice per layer.
Each iteration has an A-phase (8 L1 group-A chains + ACT stages, no DVE
dependencies - DVE backlog from the previous iteration drains under it)
and a B-phase (L1 group-B chains + SignAdd, interleaved with the previous
slices' L2/L3/L4 chains and noisy-sign ops).
"""

from contextlib import ExitStack

import numpy as np

import concourse.bass as bass  # noqa: F401
import concourse.tile as tile
from concourse import bacc, mybir
from concourse.bass_utils import run_bass_kernel_spmd

F32 = mybir.dt.float32
BF16 = mybir.dt.bfloat16
FP8 = mybir.dt.float8e4
ACTF = mybir.ActivationFunctionType
DR = mybir.MatmulPerfMode.DoubleRow

N_CORES = 8
B = 16384                 # full batch
BC = B // N_CORES         # batch per core
D_IN = 784                # layer-1 input features
D_H = 1024                # hidden features
D_OUT = 10                # output features
D_PAD4 = 16               # L4 stationary dim padded for DoubleRow
XSCALE = float(2 ** 21)   # fixed-point scale of |x|
ASCALE = float(2 ** 15)   # h1*2^21 = 2^15*A + B
KH = D_H // 128           # 8 k-chunks for hidden layers
OC = D_H // 128           # 8 output-feature chunks
# Layer-1 chunk geometry: A stream = digits (d0,d1,d2) = 2352 rows ->
# 10 DR pair-calls (20 chunks padded); B stream = (d3,d4) = 1568 rows ->
# 7 pair-calls (14 chunks padded).
KA = 20                   # A-stream chunks (incl pad), 10 pair-calls
KB = 14                   # B-stream chunks (incl pad), 7 pair-calls
KXT = KA + KB             # chunks per xt slice
# Batch-column slice widths: 512 amortizes per-op overheads and keeps DMA
# runs >= 512B; the last two slices narrow to 256 to shrink the DVE-bound
# pipeline drain (the tail's noisy-sign ops can't hide under L1 PE work).
SL = [512, 512, 512, 256, 256]
OFFS = [sum(SL[:i]) for i in range(len(SL))]
NT = len(SL)
assert sum(SL) == BC

# float32(0.5*exp(-(a*a)/50)) for a = 0,2,...,50 (bit-exact fallback table).
_PTABLE_BITS = [
    0x3F000000, 0x3EEC515A, 0x3EB9E4E3, 0x3E79375C, 0x3E0E5ACB, 0x3D8A9501,
    0x3CE5ED93, 0x3C2289CB, 0x3B43D285, 0x3A4909DD, 0x392FE09E, 0x38031DFC,
    0x36A696B8, 0x35345CD8, 0x33A6674D, 0x3202D2C5, 0x302F4A31, 0x2E4824C7,
    0x2C42BB52, 0x2A2173E9, 0x27E4229E, 0x258959AD, 0x230CEE5E, 0x207672F6,
    0x1DB79FE2, 0x1AE92B5E,
]


def _prob_table() -> np.ndarray:
    """p(a) for a = 0,2,...,50, bit-matching the reference's jnp.exp."""
    try:
        import jax.numpy as jnp

        a = np.arange(0, 51, 2, dtype=np.float32)
        p = np.asarray(0.5 * jnp.exp(-(jnp.asarray(a) * a) / (2.0 * 5.0**2)),
                       dtype=np.float32)
        if p.shape == (26,) and np.all(np.diff(p) < 0):
            return p
    except Exception:
        pass
    return np.array(_PTABLE_BITS, dtype=np.uint32).view(np.float32)


def _flip_thresholds(u: np.ndarray, ptable: np.ndarray) -> np.ndarray:
    """A(u): flip <=> |h| < A. A = 52 - 2 * #{a : p(a) <= u}."""
    tab = ptable[::-1].copy()  # ascending: p(50), p(48), ..., p(0)
    idx = np.searchsorted(tab, u, side="right")
    return (52 - 2 * idx).astype(np.int64)


def _t2_entry(A: int, fp8_np) -> float:
    """fp8 threshold T2 for even A: separates t^2 = (h'-1/4)^2 at integer
    h' into flip (|h'| < A/2) vs keep.  Must lie strictly inside
    ((A/2-0.75)^2, (A/2-0.25)^2) after fp8 rounding, with s1=32 margins."""
    if A == 0:
        return -1.0
    lo, hi = (A / 2 - 0.75) ** 2, (A / 2 - 0.25) ** 2
    cand = float(np.asarray((A / 2 - 0.5) ** 2, np.float32).astype(fp8_np))
    if not (lo < cand < hi):
        # scan fp8-representable values inside the interval
        for v in np.linspace(lo, hi, 64)[1:-1]:
            c = float(np.asarray(v, np.float32).astype(fp8_np))
            if lo < c < hi:
                cand = c
                break
        else:
            raise AssertionError(f"no fp8 threshold for A={A}")
    return cand


def _t2_table(fp8_np) -> np.ndarray:
    """T2 per A-index (A = 0,2,...,52), fp8 values as fp32, capped at 448."""
    out = np.empty(27, np.float32)
    for i in range(27):
        A = 2 * i
        if A >= 34:
            # fp8e4m3 (IEEE, max 240) can't hold larger thresholds.
            # P(A >= 34) = 6.3e-10/element: ~0.02 occurrences expected in
            # the whole problem, and an error additionally needs |h|>=32.
            out[i] = 240.0
        else:
            out[i] = _t2_entry(A, fp8_np)
    # margin check: |32*t*(t^2-T2)| >= 1 with correct sign, h' integer
    hp = np.arange(-600, 601)
    t = hp - 0.25
    for i in range(27):
        A = 2 * i
        w = 32.0 * t * (t * t - out[i])
        assert np.all(np.abs(w) >= 1.0), (A, np.abs(w).min())
        if A >= 34:
            continue
        flip = np.abs(2 * hp) < A
        s = np.where(hp > 0, 1.0, -1.0)
        want = np.where(flip, -s, s)
        assert np.all(np.clip(w, -1.0, 1.0) == want), (A,)
    return out


# ---------------------------------------------------------------------------
# Custom fused DVE ops.
#   NOISY:   out = clip(s1 * (in0-s0) * ((in0-s0)^2 - in1), -1, 1)
#   SIGNADD: out = clip((in0 + in1) * s1, -1, 1)
# ---------------------------------------------------------------------------

_NOISY_OP_NAME = "NOISY_SIGN_PM1_ANT"
_SIGNADD_OP_NAME = "SIGN_ADD_PM1_ANT"


def _noisy_ref(in0, in1, c0, c1, c2):
    t = np.asarray(in0, np.float32) - np.float32(c0)
    r2 = np.asarray(in1, np.float32).reshape(t.shape)
    w = (t * (t * t - r2)) * np.float32(c1)
    return np.maximum(np.minimum(w, np.float32(1.0)), np.float32(-1.0))


def _signadd_ref(in0, in1, c0, c1, c2):
    v = (np.asarray(in0, np.float32)
         + np.asarray(in1, np.float32).reshape(np.shape(in0)))
    w = v * np.float32(c1)
    return np.maximum(np.minimum(w, np.float32(1.0)), np.float32(-1.0))


def _register_op(name, body_fn, ref):
    from concourse import dve_ops
    from concourse.dve_spec import (C0, C1, One, Spec, Src0, Src1, Zero,
                                    lower, maxx, minn)
    from concourse.dve_uop import DveOpSpec

    for op in dve_ops.OPS:
        if op.name == name:
            return op

    body = body_fn(C0, C1, One, Zero, Src0, Src1, maxx, minn)
    spec = Spec(body=body, reference=ref)

    row = dve_ops._CUSTOM_DVE_ROW_BASE + len(dve_ops.OPS)
    assert row < 0x20, "custom-DVE opcode rows exhausted"
    shas = {}
    for ver in ("v3", "v4"):
        d = DveOpSpec(name=name, opcode=row, uops=lower(spec, ver=ver),
                      rd1_en=True)
        shas[ver] = d.sha(ver)
    op = dve_ops.DveOp(name, spec, subdim=False, uops_sha=shas)
    dve_ops.OPS.append(op)
    dve_ops.CUSTOM_DVE_SPECS[name] = spec
    dve_ops._SUB_OPCODE_FOR_NAME[name] = row
    return op


def _noisy_body(C0, C1, One, Zero, Src0, Src1, maxx, minn):
    t = Src0 - C0
    w = (t * ((t * t) - Src1)) * C1
    return maxx(minn(w, One), Zero - One)


def _signadd_body(C0, C1, One, Zero, Src0, Src1, maxx, minn):
    w = (Src0 + Src1) * C1
    return maxx(minn(w, One), Zero - One)


NOISY_OP = _register_op(_NOISY_OP_NAME, _noisy_body, _noisy_ref)
SIGNADD_OP = _register_op(_SIGNADD_OP_NAME, _signadd_body, _signadd_ref)


def build_nc(repeat: int = 1):
    """Build the per-core Bass program (same program on all 8 cores)."""
    nc = bacc.Bacc("TRN2", target_bir_lowering=False, debug=False,
                   num_devices=N_CORES)

    xt = [nc.dram_tensor(f"xt{n}", [128, KXT, SL[n]], FP8,
                         kind="ExternalInput").ap() for n in range(NT)]
    w1 = [nc.dram_tensor(f"w1_{o}", [128, KA, 128], FP8,
                         kind="ExternalInput").ap() for o in range(OC)]
    a2 = [nc.dram_tensor(f"a2_{n}", [128, OC, SL[n]], FP8,
                         kind="ExternalInput").ap() for n in range(NT)]
    a3 = [nc.dram_tensor(f"a3_{n}", [128, OC, SL[n]], FP8,
                         kind="ExternalInput").ap() for n in range(NT)]
    w2 = nc.dram_tensor("w2", [128, KH, D_H], FP8, kind="ExternalInput").ap()
    w3 = nc.dram_tensor("w3", [128, KH, D_H], FP8, kind="ExternalInput").ap()
    w4 = nc.dram_tensor("w4", [128, KH, D_PAD4], FP8,
                        kind="ExternalInput").ap()
    c1 = nc.dram_tensor("c1", [128, OC], F32, kind="ExternalInput").ap()
    out = nc.dram_tensor("out", [D_OUT, BC], F32, kind="ExternalOutput").ap()

    with tile.TileContext(nc) as tc:
        with ExitStack() as ctx:
            consts = ctx.enter_context(tc.tile_pool(name="consts", bufs=1))
            panels = ctx.enter_context(tc.tile_pool(name="panels", bufs=1))
            xtp = ctx.enter_context(tc.tile_pool(name="xtp", bufs=2))
            apool = ctx.enter_context(tc.tile_pool(name="apool", bufs=4))
            bsbp = ctx.enter_context(tc.tile_pool(name="bsbp", bufs=2))
            opool = ctx.enter_context(tc.tile_pool(name="opool", bufs=2))

            c1_t = consts.tile([128, OC], F32, tag="c1")
            w1_t = consts.tile([128, OC * KA, 128], FP8, tag="w1")
            w2_t = consts.tile([128, KH, D_H], FP8, tag="w2")
            w3_t = consts.tile([128, KH, D_H], FP8, tag="w3")
            w4_t = consts.tile([128, KH, D_PAD4], FP8, tag="w4")

            # +-1 activation panels, feature-major fp8.
            b1_t = panels.tile([128, KH, BC], FP8, tag="b1")
            b2_t = panels.tile([128, KH, BC], FP8, tag="b2")
            b3_t = panels.tile([128, KH, BC], FP8, tag="b3")

            # Priority order on the shared DMA engine: enough w1/xt to start
            # and continuously feed L1's A-phase, with the B-stream (xt
            # chunks KA..) and later consts behind it.
            # Front DMA, in deadline order on the shared DMA engine: c1
            # (stage(0) needs it early), then w1 blocks and xt0 A-chunk
            # pairs interleaved to track the PE's consumption rate, then the
            # B-stream and the remaining w1 blocks.
            xt_t: dict[int, object] = {}
            xt_t[0] = xtp.tile([128, KXT, SL[0]], FP8, tag="xt",
                               name="xt_t0")
            nc.sync.dma_start(c1_t[:], c1[:, :])
            nc.sync.dma_start(w1_t[:, 0:4, :], w1[0][:, 0:4, :])
            nc.sync.dma_start(xt_t[0][:, 0:2, :], xt[0][:, 0:2, :])
            nc.sync.dma_start(xt_t[0][:, 2:4, :], xt[0][:, 2:4, :])
            nc.sync.dma_start(w1_t[:, 4:12, :], w1[0][:, 4:12, :])
            nc.sync.dma_start(xt_t[0][:, 4:6, :], xt[0][:, 4:6, :])
            nc.sync.dma_start(xt_t[0][:, 6:8, :], xt[0][:, 6:8, :])
            nc.sync.dma_start(w1_t[:, 12:KA, :], w1[0][:, 12:, :])
            nc.sync.dma_start(xt_t[0][:, 8:10, :], xt[0][:, 8:10, :])
            nc.sync.dma_start(xt_t[0][:, 10:12, :], xt[0][:, 10:12, :])
            nc.sync.dma_start(w1_t[:, KA:KA + 10, :], w1[1][:, :10, :])
            nc.sync.dma_start(xt_t[0][:, 12:14, :], xt[0][:, 12:14, :])
            nc.sync.dma_start(w1_t[:, KA + 10:2 * KA, :], w1[1][:, 10:, :])
            nc.sync.dma_start(xt_t[0][:, 14:16, :], xt[0][:, 14:16, :])
            nc.sync.dma_start(w1_t[:, 2 * KA:2 * KA + 10, :],
                              w1[2][:, :10, :])
            nc.sync.dma_start(xt_t[0][:, 16:18, :], xt[0][:, 16:18, :])
            nc.sync.dma_start(w1_t[:, 2 * KA + 10:3 * KA, :],
                              w1[2][:, 10:, :])
            nc.sync.dma_start(xt_t[0][:, 18:KA, :], xt[0][:, 18:KA, :])
            nc.sync.dma_start(w1_t[:, 3 * KA:4 * KA, :], w1[3])
            nc.sync.dma_start(xt_t[0][:, KA:KA + 6, :], xt[0][:, KA:KA + 6, :])
            nc.sync.dma_start(w1_t[:, 4 * KA:5 * KA, :], w1[4])
            nc.sync.dma_start(xt_t[0][:, KA + 6:KA + 10, :],
                              xt[0][:, KA + 6:KA + 10, :])
            nc.sync.dma_start(w1_t[:, 5 * KA:6 * KA, :], w1[5])
            nc.sync.dma_start(xt_t[0][:, KA + 10:, :], xt[0][:, KA + 10:, :])
            nc.sync.dma_start(w1_t[:, 6 * KA:7 * KA, :], w1[6])
            nc.sync.dma_start(w1_t[:, 7 * KA:8 * KA, :], w1[7])
            nc.sync.dma_start(w4_t[:], w4[:, :, :])

            for _rep in range(repeat):
                with ExitStack() as rep_ctx:
                    l1a = rep_ctx.enter_context(
                        tc.tile_pool(name="l1a", bufs=2, space="PSUM"))
                    l1b = rep_ctx.enter_context(
                        tc.tile_pool(name="l1b", bufs=2, space="PSUM"))
                    l2ps = rep_ctx.enter_context(
                        tc.tile_pool(name="l2ps", bufs=1, space="PSUM"))
                    l3ps = rep_ctx.enter_context(
                        tc.tile_pool(name="l3ps", bufs=2, space="PSUM"))
                    l4ps = rep_ctx.enter_context(
                        tc.tile_pool(name="l4ps", bufs=1, space="PSUM"))

                    a2_t: dict[int, object] = {}
                    a3_t: dict[int, object] = {}
                    l4_pending = None  # (psum tile, slice index) across iters

                    def l4_finish(l4_fin):
                        """Last L4 DR pair + PSUM copy + output store."""
                        ps4, n4 = l4_fin
                        w4s = SL[n4]
                        s4 = slice(OFFS[n4], OFFS[n4] + w4s)
                        nc.tensor.matmul(ps4[:, :], w4_t[:, KH - 2:KH, :],
                                         b3_t[:, KH - 2:KH, s4],
                                         start=False, stop=True, perf_mode=DR)
                        ot = opool.tile([D_OUT, w4s], F32, tag="ot")
                        nc.scalar.activation(ot[:, :], ps4[:D_OUT, :],
                                             ACTF.Copy)
                        nc.sync.dma_start(out[:, s4], ot[:, :])

                    # Software pipeline, skewed one slice per layer:
                    # iteration i = [A-phase: L1-A(i) + stage | L3(i-2) +
                    # noisy3] then [B-phase: L1-B(i) + SignAdd | L2(i-1) +
                    # noisy2 | L4(i-2) pairs at odd o].  The B-phase's DVE
                    # backlog (3 ops per round vs ~1.2us of PE) drains under
                    # the next A-phase, whose L1-A chains have no DVE
                    # dependency.  L4(i-2)'s last pair + PSUM copy + store
                    # run in the next iteration's A-phase.
                    for i in range(NT + 3):
                        l4_fin, l4_pending = l4_pending, None

                        # --- DMA prefetch for this iteration ---
                        if i + 1 < NT:
                            xt_t[i + 1] = xtp.tile([128, KXT, SL[i + 1]],
                                                   FP8, tag="xt",
                                                   name=f"xt_t{i + 1}")
                            nc.sync.dma_start(xt_t[i + 1][:], xt[i + 1])
                        if i == 0:
                            nc.sync.dma_start(w2_t[:], w2[:, :, :])
                        if i == 1:
                            nc.sync.dma_start(w3_t[:], w3[:, :, :])
                        if i < NT:
                            t_a2 = apool.tile([128, OC, SL[i]], FP8,
                                              tag="a2")
                            nc.sync.dma_start(t_a2[:], a2[i])
                            a2_t[i] = t_a2
                        if 1 <= i <= NT:
                            t_a3 = apool.tile([128, OC, SL[i - 1]], FP8,
                                              tag="a3")
                            nc.sync.dma_start(t_a3[:], a3[i - 1])
                            a3_t[i - 1] = t_a3

                        n1, n2, n3 = i, i - 1, i - 2
                        if n1 < NT:
                            ws1 = SL[n1]
                            s1 = slice(OFFS[n1], OFFS[n1] + ws1)
                        if 0 <= n2 < NT:
                            ws2 = SL[n2]
                            s2 = slice(OFFS[n2], OFFS[n2] + ws2)
                        if 0 <= n3 < NT:
                            ws3 = SL[n3]
                            s3 = slice(OFFS[n3], OFFS[n3] + ws3)

                        # --- A-phase: L1-A chains + stages | L3 + noisy3 ---
                        bsa = None
                        if n1 < NT:
                            bsa = bsbp.tile([128, OC, ws1], F32, tag="bsb")
                        for o in range(OC):
                            if n1 < NT:
                                pa = l1a.tile([128, ws1], F32, tag="mma")
                                for kp in range(KA // 2):
                                    nc.tensor.matmul(
                                        pa[:, :],
                                        w1_t[:, o * KA + 2 * kp:
                                             o * KA + 2 * kp + 2, :],
                                        xt_t[n1][:, 2 * kp:2 * kp + 2, :],
                                        start=(kp == 0),
                                        stop=(kp == KA // 2 - 1),
                                        perf_mode=DR,
                                    )
                                # bsb = 2^15*A + c1 (c1 = -2^21*mu1)
                                nc.scalar.activation(
                                    bsa[:, o, :], pa[:, :], ACTF.Identity,
                                    scale=ASCALE, bias=c1_t[:, o:o + 1])
                            if o == 3 and l4_fin is not None:
                                l4_finish(l4_fin)
                                l4_fin = None
                            if 0 <= n3 < NT:
                                p3 = l3ps.tile([128, ws3], F32, tag="mm3")
                                for kp in range(KH // 2):
                                    nc.tensor.matmul(
                                        p3[:, :],
                                        w3_t[:, 2 * kp:2 * kp + 2,
                                             o * 128:(o + 1) * 128],
                                        b2_t[:, 2 * kp:2 * kp + 2, s3],
                                        start=(kp == 0),
                                        stop=(kp == KH // 2 - 1),
                                        perf_mode=DR,
                                    )
                                nc.vector._custom_dve(
                                    NOISY_OP, out=b3_t[:, o, s3],
                                    in0=p3[:, :], in1=a3_t[n3][:, o, :],
                                    s0=0.25, s1=32.0)
                        if l4_fin is not None:  # iterations with empty loop
                            l4_finish(l4_fin)

                        # --- B-phase: L1-B + SignAdd | L2 + noisy2 | L4 ---
                        for o in range(OC):
                            if n1 < NT:
                                pb = l1b.tile([128, ws1], F32, tag="mmb")
                                for kp in range(KB // 2):
                                    nc.tensor.matmul(
                                        pb[:, :],
                                        w1_t[:, o * KA + 2 * kp:
                                             o * KA + 2 * kp + 2, :],
                                        xt_t[n1][:, KA + 2 * kp:
                                                 KA + 2 * kp + 2, :],
                                        start=(kp == 0),
                                        stop=(kp == KB // 2 - 1),
                                        perf_mode=DR,
                                    )
                                # b1 = clip((B + bsb)*2^35) = sign(h1-mu1)
                                nc.vector._custom_dve(
                                    SIGNADD_OP, out=b1_t[:, o, s1],
                                    in0=pb[:, :], in1=bsa[:, o, :],
                                    s0=0.0, s1=float(2.0 ** 35))
                            if 0 <= n3 < NT and o >= 3 and o % 2 == 1:
                                kp = (o - 3) // 2
                                if kp == 0:
                                    ps4 = l4ps.tile([D_PAD4, ws3], F32,
                                                    tag="mm4")
                                    l4_pending = (ps4, n3)
                                nc.tensor.matmul(
                                    ps4[:, :],
                                    w4_t[:, 2 * kp:2 * kp + 2, :],
                                    b3_t[:, 2 * kp:2 * kp + 2, s3],
                                    start=(kp == 0), stop=False,
                                    perf_mode=DR)
                            if 0 <= n2 < NT - 1:
                                p2 = l2ps.tile([128, ws2], F32, tag="mm2")
                                for kp in range(KH // 2):
                                    nc.tensor.matmul(
                                        p2[:, :],
                                        w2_t[:, 2 * kp:2 * kp + 2,
                                             o * 128:(o + 1) * 128],
                                        b1_t[:, 2 * kp:2 * kp + 2, s2],
                                        start=(kp == 0),
                                        stop=(kp == KH // 2 - 1),
                                        perf_mode=DR,
                                    )
                                nc.vector._custom_dve(
                                    NOISY_OP, out=b2_t[:, o, s2],
                                    in0=p2[:, :], in1=a2_t[n2][:, o, :],
                                    s0=0.25, s1=32.0)

                        # --- gamma-phase (last L1 iteration only): run the
                        # final slice's L2 immediately, so its noisy-sign DVE
                        # work overlaps the remaining tail PE work instead of
                        # serializing after it.
                        if i == NT - 1:
                            sg = s1
                            for o in range(OC):
                                # reuse the (now idle) double-buffered L1-A
                                # banks so gamma never waits on its noisy ops
                                p2 = l1a.tile([128, ws1], F32, tag="mma")
                                for kp in range(KH // 2):
                                    nc.tensor.matmul(
                                        p2[:, :],
                                        w2_t[:, 2 * kp:2 * kp + 2,
                                             o * 128:(o + 1) * 128],
                                        b1_t[:, 2 * kp:2 * kp + 2, sg],
                                        start=(kp == 0),
                                        stop=(kp == KH // 2 - 1),
                                        perf_mode=DR,
                                    )
                                nc.vector._custom_dve(
                                    NOISY_OP, out=b2_t[:, o, sg],
                                    in0=p2[:, :], in1=a2_t[n1][:, o, :],
                                    s0=0.25, s1=32.0)

    nc.compile()
    return nc


_NC_CACHE: dict[int, object] = {}


def _get_nc(repeat: int = 1):
    if repeat not in _NC_CACHE:
        _NC_CACHE[repeat] = build_nc(repeat)
    return _NC_CACHE[repeat]


def make_in_maps(x, u2, u3, W1, W2, W3, W4, **_unused):
    """Host preprocessing -> per-core input dicts."""
    fp8_np = mybir.dt.np(FP8)

    x = np.ascontiguousarray(np.asarray(x, dtype=np.float32))
    W1b = np.sign(np.asarray(W1, dtype=np.float32))
    # mean(h1, axis=0) = sign(W1) @ mean(x, axis=0), in float64; negated and
    # pre-scaled so the device computes sign(2^21 h1 + bias).
    mu1 = (W1b.astype(np.float64) @ x.mean(axis=0, dtype=np.float64)).astype(
        np.float32)
    c1 = np.ascontiguousarray(
        (np.float32(-XSCALE) * mu1).reshape(OC, 128).T)  # [128, OC]

    # balanced base-32 fixed-point split: round(x*2^21) = sum d_j 32^(4-j),
    # digits d_j in [-16, 16] - every one exact in fp8e4m3.
    assert np.abs(x).max() * XSCALE < 16.5 * (32 ** 5 - 1) / 31, "x overflow"
    n = np.rint(x.T.astype(np.float64) * XSCALE).astype(np.int64)  # [784, B]
    digs = []
    for _ in range(5):
        d = ((n + 16) % 32) - 16
        n = (n - d) >> 5
        digs.append(d)                      # LSB first: d4, d3, d2, d1, d0
    assert np.all(n == 0), "digit overflow"
    d4, d3, d2, d1, d0 = digs

    # xt slice layout [128, KXT, SW]: A stream (d0,d1,d2 = 2352 rows, pad
    # to KA*128) then B stream (d3,d4 = 1568 rows, pad to KB*128).
    xt_all = np.zeros((KXT * 128, B), dtype=fp8_np)
    for j, d in enumerate((d0, d1, d2)):
        xt_all[j * D_IN:(j + 1) * D_IN] = d.astype(np.float32).astype(fp8_np)
    for j, d in enumerate((d3, d4)):
        r = KA * 128 + j * D_IN
        xt_all[r:r + D_IN] = d.astype(np.float32).astype(fp8_np)

    # stored weight stream per out-chunk: [32*w, w, w/32] (2352 rows + pad),
    # shared by groups A and B (B reads the 2/3 prefix).
    w1p = np.zeros((KA * 128, D_H), dtype=fp8_np)
    for j, sc in enumerate((32.0, 1.0, 1.0 / 32.0)):
        w1p[j * D_IN:(j + 1) * D_IN] = (
            W1b.T * np.float32(sc)).astype(fp8_np)
    # [o][p][k][m]: one contiguous DMA per 128-feature output block.
    w1_blocks = np.ascontiguousarray(
        w1p.reshape(KA, 128, OC, 128).transpose(2, 1, 0, 3))

    pt = _prob_table()
    t2tab = _t2_table(fp8_np)
    a2f = t2tab[(_flip_thresholds(np.asarray(u2), pt) // 2)]   # [B, 1024]
    a3f = t2tab[(_flip_thresholds(np.asarray(u3), pt) // 2)]

    def _hidden_w(w, sc):
        wt = (np.sign(np.asarray(w, np.float32)) * np.float32(sc)
              ).T.astype(fp8_np)                               # [K, M]
        return np.ascontiguousarray(
            wt.reshape(KH, 128, wt.shape[1]).transpose(1, 0, 2))

    w2t = _hidden_w(W2, 0.5)               # [128, 8, 1024], +-0.5
    w3t = _hidden_w(W3, 0.5)
    w4t = _hidden_w(W4, 1.0)               # [128, 8, 10]
    w4p = np.zeros((128, KH, D_PAD4), dtype=fp8_np)
    w4p[:, :, :D_OUT] = w4t
    w4t = w4p

    in_maps = []
    for c in range(N_CORES):
        sl = slice(c * BC, (c + 1) * BC)
        m = {"w2": w2t, "w3": w3t, "w4": w4t, "c1": c1}
        xc = xt_all[:, sl].reshape(KXT, 128, BC)  # [k, p, col]
        for nn in range(NT):
            cs = slice(OFFS[nn], OFFS[nn] + SL[nn])
            m[f"xt{nn}"] = np.ascontiguousarray(
                xc[:, :, cs].transpose(1, 0, 2))
        for o in range(OC):
            m[f"w1_{o}"] = w1_blocks[o]
        for nm, tab in (("a2", a2f), ("a3", a3f)):
            rc = tab.T[:, sl].astype(fp8_np).reshape(OC, 128, BC)
            for nn in range(NT):
                cs = slice(OFFS[nn], OFFS[nn] + SL[nn])
                m[f"{nm}_{nn}"] = np.ascontiguousarray(
                    rc[:, :, cs].transpose(1, 0, 2))
        in_maps.append(m)
    return in_maps


def kernel(x, u2, u3, W1, W2, W3, W4,
           g1=None, b1=None, g2=None, b2=None, g3=None, b3=None):
    for g in (g1, g2, g3):
        assert g is None or np.all(np.asarray(g) > 0), "kernel assumes g > 0"
    for b in (b1, b2, b3):
        assert b is None or np.all(np.asarray(b) == 0), "kernel assumes b == 0"

    nc = _get_nc(repeat=1)
    in_maps = make_in_maps(x, u2, u3, W1, W2, W3, W4)
    res = run_bass_kernel_spmd(nc, in_maps, core_ids=list(range(N_CORES)))

    out = np.empty((B, D_OUT), dtype=np.float32)
    for c in range(N_CORES):
        out[c * BC:(c + 1) * BC, :] = res.results[c]["out"].T
    return out


# revision 29
# speedup vs baseline: 1.0320x; 1.0320x over previous
"""Bass/Trainium2 kernel for a binarized NN (BNN) forward pass, data-parallel
over 8 NeuronCores.

Reference semantics (fp32):
    h1 = x @ sign(W1).T;  b1 = sign(h1 - mean(h1, axis=0))        # g=1, b=0
    h2 = b1 @ sign(W2).T; b2 = noisy_sign(h2, u2)                  # BN+sign is
    h3 = b2 @ sign(W3).T; b3 = noisy_sign(h3, u3)                  # identity on +-1
    out = b3 @ sign(W4).T

Implementation summary (all arithmetic exact small-integer, as in the
fixed-point analysis below):
  * Layer 1 contracts a balanced base-32 fixed-point split of x:
    round(x*2^21) = sum_j d_j 32^(4-j) with digits d_j in [-16,16], every
    one exact in fp8e4m3.  Two PSUM groups share one stored weight stream
    [32*w, w, w/32] (w = sign(W1)): group A moves digits (d0,d1,d2) over
    the full stream, group B moves (d3,d4) over its 2/3 prefix, so
    h1*2^21 = 2^15*A + B.  All products and DoubleRow pair-sums are
    integers scaled by powers of two within the pair-adder's ~12-bit
    mantissa, and each group's PSUM total stays under 2^24/granularity,
    so accumulation is exact.  17 DR pair-matmuls per output chunk
    (vs 20 for the 6-nibble scheme).
  * ACT stages bsb = 2^15*A + c1 (c1 = -2^21*mean(h1), computed on host in
    float64); a single custom DVE op then emits
    b1 = clip((B + bsb) * 2^35, -1, 1) = sign(h1 - mu1) directly.  The
    only inexactness is one fp32 rounding in the stage/add (~2^-19 of h1)
    plus the 2^-22 input quantization - both far below the reference's
    own matmul rounding scale.
  * W2/W3 ship as +-0.5 so PSUM holds h' = h/2 (an exact integer <= 512).
    The stochastic flip (u < 0.5 exp(-h^2/50), |h| <= 50) depends only on
    A(u) = smallest even a with p(a) <= u: flip <=> |h| < A.  With
    t = h' - 1/4 and T2 ~ ((A-1)/2)^2 (fp8-rounded inside its decision
    margin, -1 when A = 0),
        noisy = clip(32 * t * (t*t - T2), -1, 1)
    equals the exact noisy sign for every integer h, so the u-derived
    tables ship as fp8, halving their HBM traffic vs bf16.
  * b in {+-1} and sign(W4) in {+-1} make the output exact integers.

Layout is feature-major: activations live as [features(partitions),
batch(free)].  Batch 16384 is sharded 2048/core; each core pipelines four
512-column slices through all four layers, skewed one slice per layer.
Each iteration has an A-phase (8 L1 group-A chains + ACT stages, no DVE
dependencies - DVE backlog from the previous iteration drains under it)
and a B-phase (L1 group-B chains + SignAdd, interleaved with the previous
slices' L2/L3/L4 chains and noisy-sign ops).
"""

from contextlib import ExitStack

import numpy as np

import concourse.bass as bass  # noqa: F401
import concourse.tile as tile
from concourse import bacc, mybir
from concourse.bass_utils import run_bass_kernel_spmd

F32 = mybir.dt.float32
BF16 = mybir.dt.bfloat16
FP8 = mybir.dt.float8e4
ACTF = mybir.ActivationFunctionType
DR = mybir.MatmulPerfMode.DoubleRow

N_CORES = 8
B = 16384                 # full batch
BC = B // N_CORES         # batch per core
D_IN = 784                # layer-1 input features
D_H = 1024                # hidden features
D_OUT = 10                # output features
D_PAD4 = 16               # L4 stationary dim padded for DoubleRow
XSCALE = float(2 ** 21)   # fixed-point scale of |x|
ASCALE = float(2 ** 15)   # h1*2^21 = 2^15*A + B
KH = D_H // 128           # 8 k-chunks for hidden layers
OC = D_H // 128           # 8 output-feature chunks
# Layer-1 chunk geometry: A stream = digits (d0,d1,d2) = 2352 rows ->
# 10 DR pair-calls (20 chunks padded); B stream = (d3,d4) = 1568 rows ->
# 7 pair-calls (14 chunks padded).
KA = 20                   # A-stream chunks (incl pad), 10 pair-calls
KB = 14                   # B-stream chunks (incl pad), 7 pair-calls
KXT = KA + KB             # chunks per xt slice
# Batch-column slice widths: 512 amortizes per-op overheads and keeps DMA
# runs >= 512B; the last two slices narrow to 256 to shrink the DVE-bound
# pipeline drain (the tail's noisy-sign ops can't hide under L1 PE work).
SL = [512, 512, 512, 256, 256]
OFFS = [sum(SL[:i]) for i in range(len(SL))]
NT = len(SL)
assert sum(SL) == BC

# float32(0.5*exp(-(a*a)/50)) for a = 0,2,...,50 (bit-exact fallback table).
_PTABLE_BITS = [
    0x3F000000, 0x3EEC515A, 0x3EB9E4E3, 0x3E79375C, 0x3E0E5ACB, 0x3D8A9501,
    0x3CE5ED93, 0x3C2289CB, 0x3B43D285, 0x3A4909DD, 0x392FE09E, 0x38031DFC,
    0x36A696B8, 0x35345CD8, 0x33A6674D, 0x3202D2C5, 0x302F4A31, 0x2E4824C7,
    0x2C42BB52, 0x2A2173E9, 0x27E4229E, 0x258959AD, 0x230CEE5E, 0x207672F6,
    0x1DB79FE2, 0x1AE92B5E,
]


def _prob_table() -> np.ndarray:
    """p(a) for a = 0,2,...,50, bit-matching the reference's jnp.exp."""
    try:
        import jax.numpy as jnp

        a = np.arange(0, 51, 2, dtype=np.float32)
        p = np.asarray(0.5 * jnp.exp(-(jnp.asarray(a) * a) / (2.0 * 5.0**2)),
                       dtype=np.float32)
        if p.shape == (26,) and np.all(np.diff(p) < 0):
            return p
    except Exception:
        pass
    return np.array(_PTABLE_BITS, dtype=np.uint32).view(np.float32)


def _flip_thresholds(u: np.ndarray, ptable: np.ndarray) -> np.ndarray:
    """A(u): flip <=> |h| < A. A = 52 - 2 * #{a : p(a) <= u}."""
    tab = ptable[::-1].copy()  # ascending: p(50), p(48), ..., p(0)
    idx = np.searchsorted(tab, u, side="right")
    return (52 - 2 * idx).astype(np.int64)


def _t2_entry(A: int, fp8_np) -> float:
    """fp8 threshold T2 for even A: separates t^2 = (h'-1/4)^2 at integer
    h' into flip (|h'| < A/2) vs keep.  Must lie strictly inside
    ((A/2-0.75)^2, (A/2-0.25)^2) after fp8 rounding, with s1=32 margins."""
    if A == 0:
        return -1.0
    lo, hi = (A / 2 - 0.75) ** 2, (A / 2 - 0.25) ** 2
    cand = float(np.asarray((A / 2 - 0.5) ** 2, np.float32).astype(fp8_np))
    if not (lo < cand < hi):
        # scan fp8-representable values inside the interval
        for v in np.linspace(lo, hi, 64)[1:-1]:
            c = float(np.asarray(v, np.float32).astype(fp8_np))
            if lo < c < hi:
                cand = c
                break
        else:
            raise AssertionError(f"no fp8 threshold for A={A}")
    return cand


def _t2_table(fp8_np) -> np.ndarray:
    """T2 per A-index (A = 0,2,...,52), fp8 values as fp32, capped at 448."""
    out = np.empty(27, np.float32)
    for i in range(27):
        A = 2 * i
        if A >= 34:
            # fp8e4m3 (IEEE, max 240) can't hold larger thresholds.
            # P(A >= 34) = 6.3e-10/element: ~0.02 occurrences expected in
            # the whole problem, and an error additionally needs |h|>=32.
            out[i] = 240.0
        else:
            out[i] = _t2_entry(A, fp8_np)
    # margin check: |32*t*(t^2-T2)| >= 1 with correct sign, h' integer
    hp = np.arange(-600, 601)
    t = hp - 0.25
    for i in range(27):
        A = 2 * i
        w = 32.0 * t * (t * t - out[i])
        assert np.all(np.abs(w) >= 1.0), (A, np.abs(w).min())
        if A >= 34:
            continue
        flip = np.abs(2 * hp) < A
        s = np.where(hp > 0, 1.0, -1.0)
        want = np.where(flip, -s, s)
        assert np.all(np.clip(w, -1.0, 1.0) == want), (A,)
    return out


# ---------------------------------------------------------------------------
# Custom fused DVE ops.
#   NOISY:   out = clip(s1 * (in0-s0) * ((in0-s0)^2 - in1), -1, 1)
#   SIGNADD: out = clip((in0 + in1) * s1, -1, 1)
# ---------------------------------------------------------------------------

_NOISY_OP_NAME = "NOISY_SIGN_PM1_ANT"
_SIGNADD_OP_NAME = "SIGN_ADD_PM1_ANT"


def _noisy_ref(in0, in1, c0, c1, c2):
    t = np.asarray(in0, np.float32) - np.float32(c0)
    r2 = np.asarray(in1, np.float32).reshape(t.shape)
    w = (t * (t * t - r2)) * np.float32(c1)
    return np.maximum(np.minimum(w, np.float32(1.0)), np.float32(-1.0))


def _signadd_ref(in0, in1, c0, c1, c2):
    v = (np.asarray(in0, np.float32)
         + np.asarray(in1, np.float32).reshape(np.shape(in0)))
    w = v * np.float32(c1)
    return np.maximum(np.minimum(w, np.float32(1.0)), np.float32(-1.0))


def _register_op(name, body_fn, ref):
    from concourse import dve_ops
    from concourse.dve_spec import (C0, C1, One, Spec, Src0, Src1, Zero,
                                    lower, maxx, minn)
    from concourse.dve_uop import DveOpSpec

    for op in dve_ops.OPS:
        if op.name == name:
            return op

    body = body_fn(C0, C1, One, Zero, Src0, Src1, maxx, minn)
    spec = Spec(body=body, reference=ref)

    row = dve_ops._CUSTOM_DVE_ROW_BASE + len(dve_ops.OPS)
    assert row < 0x20, "custom-DVE opcode rows exhausted"
    shas = {}
    for ver in ("v3", "v4"):
        d = DveOpSpec(name=name, opcode=row, uops=lower(spec, ver=ver),
                      rd1_en=True)
        shas[ver] = d.sha(ver)
    op = dve_ops.DveOp(name, spec, subdim=False, uops_sha=shas)
    dve_ops.OPS.append(op)
    dve_ops.CUSTOM_DVE_SPECS[name] = spec
    dve_ops._SUB_OPCODE_FOR_NAME[name] = row
    return op


def _noisy_body(C0, C1, One, Zero, Src0, Src1, maxx, minn):
    t = Src0 - C0
    w = (t * ((t * t) - Src1)) * C1
    return maxx(minn(w, One), Zero - One)


def _signadd_body(C0, C1, One, Zero, Src0, Src1, maxx, minn):
    w = (Src0 + Src1) * C1
    return maxx(minn(w, One), Zero - One)


NOISY_OP = _register_op(_NOISY_OP_NAME, _noisy_body, _noisy_ref)
SIGNADD_OP = _register_op(_SIGNADD_OP_NAME, _signadd_body, _signadd_ref)


def build_nc(repeat: int = 1):
    """Build the per-core Bass program (same program on all 8 cores)."""
    nc = bacc.Bacc("TRN2", target_bir_lowering=False, debug=False,
                   num_devices=N_CORES)

    xt = [nc.dram_tensor(f"xt{n}", [128, KXT, SL[n]], FP8,
                         kind="ExternalInput").ap() for n in range(NT)]
    w1 = [nc.dram_tensor(f"w1_{o}", [128, KA, 128], FP8,
                         kind="ExternalInput").ap() for o in range(OC)]
    a2 = [nc.dram_tensor(f"a2_{n}", [128, OC, SL[n]], FP8,
                         kind="ExternalInput").ap() for n in range(NT)]
    a3 = [nc.dram_tensor(f"a3_{n}", [128, OC, SL[n]], FP8,
                         kind="ExternalInput").ap() for n in range(NT)]
    w2 = nc.dram_tensor("w2", [128, KH, D_H], FP8, kind="ExternalInput").ap()
    w3 = nc.dram_tensor("w3", [128, KH, D_H], FP8, kind="ExternalInput").ap()
    w4 = nc.dram_tensor("w4", [128, KH, D_PAD4], FP8,
                        kind="ExternalInput").ap()
    c1 = nc.dram_tensor("c1", [128, OC], F32, kind="ExternalInput").ap()
    out = nc.dram_tensor("out", [D_OUT, BC], F32, kind="ExternalOutput").ap()

    with tile.TileContext(nc) as tc:
        with ExitStack() as ctx:
            consts = ctx.enter_context(tc.tile_pool(name="consts", bufs=1))
            panels = ctx.enter_context(tc.tile_pool(name="panels", bufs=1))
            xtp = ctx.enter_context(tc.tile_pool(name="xtp", bufs=2))
            apool = ctx.enter_context(tc.tile_pool(name="apool", bufs=4))
            opool = ctx.enter_context(tc.tile_pool(name="opool", bufs=2))

            c1_t = consts.tile([128, OC], F32, tag="c1")
            w1_t = consts.tile([128, OC * KA, 128], FP8, tag="w1")
            w2_t = consts.tile([128, KH, D_H], FP8, tag="w2")
            w3_t = consts.tile([128, KH, D_H], FP8, tag="w3")
            w4_t = consts.tile([128, KH, D_PAD4], FP8, tag="w4")

            # +-1 activation panels, feature-major fp8.
            b1_t = panels.tile([128, KH, BC], FP8, tag="b1")
            b2_t = panels.tile([128, KH, BC], FP8, tag="b2")
            b3_t = panels.tile([128, KH, BC], FP8, tag="b3")

            # Priority order on the shared DMA engine: enough w1/xt to start
            # and continuously feed L1's A-phase, with the B-stream (xt
            # chunks KA..) and later consts behind it.
            # Front DMA, in deadline order on the shared DMA engine: c1
            # (stage(0) needs it early), then w1 blocks and xt0 A-chunk
            # pairs interleaved to track the PE's consumption rate, then the
            # B-stream and the remaining w1 blocks.
            xt_t: dict[int, object] = {}
            xt_t[0] = xtp.tile([128, KXT, SL[0]], FP8, tag="xt",
                               name="xt_t0")
            nc.sync.dma_start(c1_t[:], c1[:, :])
            nc.sync.dma_start(w1_t[:, 0:4, :], w1[0][:, 0:4, :])
            nc.sync.dma_start(xt_t[0][:, 0:2, :], xt[0][:, 0:2, :])
            nc.sync.dma_start(xt_t[0][:, 2:4, :], xt[0][:, 2:4, :])
            nc.sync.dma_start(w1_t[:, 4:12, :], w1[0][:, 4:12, :])
            nc.sync.dma_start(xt_t[0][:, 4:6, :], xt[0][:, 4:6, :])
            nc.sync.dma_start(xt_t[0][:, 6:8, :], xt[0][:, 6:8, :])
            nc.sync.dma_start(w1_t[:, 12:KA, :], w1[0][:, 12:, :])
            nc.sync.dma_start(xt_t[0][:, 8:10, :], xt[0][:, 8:10, :])
            nc.sync.dma_start(xt_t[0][:, 10:12, :], xt[0][:, 10:12, :])
            nc.sync.dma_start(w1_t[:, KA:KA + 10, :], w1[1][:, :10, :])
            nc.sync.dma_start(xt_t[0][:, 12:14, :], xt[0][:, 12:14, :])
            nc.sync.dma_start(w1_t[:, KA + 10:2 * KA, :], w1[1][:, 10:, :])
            nc.sync.dma_start(xt_t[0][:, 14:16, :], xt[0][:, 14:16, :])
            nc.sync.dma_start(w1_t[:, 2 * KA:2 * KA + 10, :],
                              w1[2][:, :10, :])
            nc.sync.dma_start(xt_t[0][:, 16:18, :], xt[0][:, 16:18, :])
            nc.sync.dma_start(w1_t[:, 2 * KA + 10:3 * KA, :],
                              w1[2][:, 10:, :])
            nc.sync.dma_start(xt_t[0][:, 18:KA, :], xt[0][:, 18:KA, :])
            nc.sync.dma_start(w1_t[:, 3 * KA:4 * KA, :], w1[3])
            nc.sync.dma_start(xt_t[0][:, KA:KA + 6, :], xt[0][:, KA:KA + 6, :])
            nc.sync.dma_start(w1_t[:, 4 * KA:5 * KA, :], w1[4])
            nc.sync.dma_start(xt_t[0][:, KA + 6:KA + 10, :],
                              xt[0][:, KA + 6:KA + 10, :])
            nc.sync.dma_start(w1_t[:, 5 * KA:6 * KA, :], w1[5])
            nc.sync.dma_start(xt_t[0][:, KA + 10:, :], xt[0][:, KA + 10:, :])
            nc.sync.dma_start(w1_t[:, 6 * KA:7 * KA, :], w1[6])
            nc.sync.dma_start(w1_t[:, 7 * KA:8 * KA, :], w1[7])
            nc.sync.dma_start(w4_t[:], w4[:, :, :])

            for _rep in range(repeat):
                with ExitStack() as rep_ctx:
                    l1a = rep_ctx.enter_context(
                        tc.tile_pool(name="l1a", bufs=2, space="PSUM"))
                    l1b = rep_ctx.enter_context(
                        tc.tile_pool(name="l1b", bufs=2, space="PSUM"))
                    l23 = rep_ctx.enter_context(
                        tc.tile_pool(name="l23", bufs=3, space="PSUM"))
                    l4ps = rep_ctx.enter_context(
                        tc.tile_pool(name="l4ps", bufs=1, space="PSUM"))

                    a2_t: dict[int, object] = {}
                    a3_t: dict[int, object] = {}
                    l4_pending = None  # (psum tile, slice index) across iters

                    def l4_finish(l4_fin):
                        """Last L4 DR pair + PSUM copy + output store."""
                        ps4, n4 = l4_fin
                        w4s = SL[n4]
                        s4 = slice(OFFS[n4], OFFS[n4] + w4s)
                        nc.tensor.matmul(ps4[:, :], w4_t[:, KH - 2:KH, :],
                                         b3_t[:, KH - 2:KH, s4],
                                         start=False, stop=True, perf_mode=DR)
                        ot = opool.tile([D_OUT, w4s], F32, tag="ot")
                        nc.scalar.activation(ot[:, :], ps4[:D_OUT, :],
                                             ACTF.Copy)
                        nc.sync.dma_start(out[:, s4], ot[:, :])

                    # Software pipeline, skewed one slice per layer.  Each
                    # iteration i runs 8 uniform rounds (one per out-chunk):
                    #   A(i,o) -> stage -> B(i,o-1) -> Sign | L2(i-1,o) +
                    #   noisy2 | L3(i-2,o) + noisy3 | L4(i-2) pair at odd o
                    # The stage writes 2^15*A + c1 INTO the group-B PSUM
                    # bank; the B pair-matmuls then accumulate on top
                    # (start=False), so b1 = Sign(PSUM) is a plain ACT op
                    # and the DVE runs only the two noisy-sign ops per
                    # round (~1.3us DVE vs ~2.7us PE: always PE-bound).
                    # B trails A by one round so the stage (ACT) hides
                    # under A(o+1).  L4(i-2)'s last pair + PSUM copy +
                    # store run in the next iteration's round 4.
                    for i in range(NT + 3):
                        l4_fin, l4_pending = l4_pending, None

                        # --- DMA prefetch for this iteration ---
                        if i + 1 < NT:
                            xt_t[i + 1] = xtp.tile([128, KXT, SL[i + 1]],
                                                   FP8, tag="xt",
                                                   name=f"xt_t{i + 1}")
                            nc.sync.dma_start(xt_t[i + 1][:], xt[i + 1])
                        if i == 0:
                            nc.sync.dma_start(w2_t[:], w2[:, :, :])
                        if i == 1:
                            nc.sync.dma_start(w3_t[:], w3[:, :, :])
                        if i < NT:
                            t_a2 = apool.tile([128, OC, SL[i]], FP8,
                                              tag="a2")
                            nc.sync.dma_start(t_a2[:], a2[i])
                            a2_t[i] = t_a2
                        if 1 <= i <= NT:
                            t_a3 = apool.tile([128, OC, SL[i - 1]], FP8,
                                              tag="a3")
                            nc.sync.dma_start(t_a3[:], a3[i - 1])
                            a3_t[i - 1] = t_a3

                        n1, n2, n3 = i, i - 1, i - 2
                        if n1 < NT:
                            ws1 = SL[n1]
                            s1 = slice(OFFS[n1], OFFS[n1] + ws1)
                        if 0 <= n2 < NT:
                            ws2 = SL[n2]
                            s2 = slice(OFFS[n2], OFFS[n2] + ws2)
                        if 0 <= n3 < NT:
                            ws3 = SL[n3]
                            s3 = slice(OFFS[n3], OFFS[n3] + ws3)

                        def l1_b_chain(o, pb):
                            """Group-B pairs accumulate onto the staged
                            PSUM bank (start=False), then b1 = Sign(PSUM)."""
                            for kp in range(KB // 2):
                                # start=False on an ACT-initialized bank is
                                # intentional: accumulate onto 2^15*A + c1.
                                nc.tensor.matmul(
                                    pb[:, :],
                                    w1_t[:, o * KA + 2 * kp:
                                         o * KA + 2 * kp + 2, :],
                                    xt_t[n1][:, KA + 2 * kp:
                                             KA + 2 * kp + 2, :],
                                    start=False,
                                    stop=(kp == KB // 2 - 1),
                                    perf_mode=DR,
                                    skip_group_check=True,
                                )
                            nc.scalar.activation(
                                b1_t[:, o, s1], pb[:, :], ACTF.Sign)

                        pb_prev = None  # (o, staged PSUM tile) from last round
                        for o in range(OC):
                            if n1 < NT:
                                pa = l1a.tile([128, ws1], F32, tag="mma")
                                for kp in range(KA // 2):
                                    nc.tensor.matmul(
                                        pa[:, :],
                                        w1_t[:, o * KA + 2 * kp:
                                             o * KA + 2 * kp + 2, :],
                                        xt_t[n1][:, 2 * kp:2 * kp + 2, :],
                                        start=(kp == 0),
                                        stop=(kp == KA // 2 - 1),
                                        perf_mode=DR,
                                    )
                                # PSUM init for group B: 2^15*A + c1
                                pb = l1b.tile([128, ws1], F32, tag="mmb")
                                nc.scalar.activation(
                                    pb[:, :], pa[:, :], ACTF.Identity,
                                    scale=ASCALE, bias=c1_t[:, o:o + 1])
                                if pb_prev is not None:
                                    l1_b_chain(*pb_prev)
                                pb_prev = (o, pb)
                            if o == 3 and l4_fin is not None:
                                l4_finish(l4_fin)
                                l4_fin = None
                            if 0 <= n2 < NT - 1:
                                p2 = l23.tile([128, ws2], F32, tag="mm23",
                                              name="p2")
                                for kp in range(KH // 2):
                                    nc.tensor.matmul(
                                        p2[:, :],
                                        w2_t[:, 2 * kp:2 * kp + 2,
                                             o * 128:(o + 1) * 128],
                                        b1_t[:, 2 * kp:2 * kp + 2, s2],
                                        start=(kp == 0),
                                        stop=(kp == KH // 2 - 1),
                                        perf_mode=DR,
                                    )
                                nc.vector._custom_dve(
                                    NOISY_OP, out=b2_t[:, o, s2],
                                    in0=p2[:, :], in1=a2_t[n2][:, o, :],
                                    s0=0.25, s1=32.0)
                            if 0 <= n3 < NT:
                                p3 = l23.tile([128, ws3], F32, tag="mm23",
                                              name="p3")
                                for kp in range(KH // 2):
                                    nc.tensor.matmul(
                                        p3[:, :],
                                        w3_t[:, 2 * kp:2 * kp + 2,
                                             o * 128:(o + 1) * 128],
                                        b2_t[:, 2 * kp:2 * kp + 2, s3],
                                        start=(kp == 0),
                                        stop=(kp == KH // 2 - 1),
                                        perf_mode=DR,
                                    )
                                nc.vector._custom_dve(
                                    NOISY_OP, out=b3_t[:, o, s3],
                                    in0=p3[:, :], in1=a3_t[n3][:, o, :],
                                    s0=0.25, s1=32.0)
                                if o >= 3 and o % 2 == 1:
                                    kp = (o - 3) // 2
                                    if kp == 0:
                                        ps4 = l4ps.tile([D_PAD4, ws3], F32,
                                                        tag="mm4")
                                        l4_pending = (ps4, n3)
                                    nc.tensor.matmul(
                                        ps4[:, :],
                                        w4_t[:, 2 * kp:2 * kp + 2, :],
                                        b3_t[:, 2 * kp:2 * kp + 2, s3],
                                        start=(kp == 0), stop=False,
                                        perf_mode=DR)
                        if pb_prev is not None:
                            l1_b_chain(*pb_prev)
                        if l4_fin is not None:  # iterations with empty loop
                            l4_finish(l4_fin)

                        # --- gamma-phase (last L1 iteration only): run the
                        # final slice's L2 immediately, so its noisy-sign DVE
                        # work overlaps the remaining tail PE work instead of
                        # serializing after it.
                        if i == NT - 1:
                            sg = s1
                            for o in range(OC):
                                p2 = l23.tile([128, ws1], F32, tag="mm23",
                                              name="p2g")
                                for kp in range(KH // 2):
                                    nc.tensor.matmul(
                                        p2[:, :],
                                        w2_t[:, 2 * kp:2 * kp + 2,
                                             o * 128:(o + 1) * 128],
                                        b1_t[:, 2 * kp:2 * kp + 2, sg],
                                        start=(kp == 0),
                                        stop=(kp == KH // 2 - 1),
                                        perf_mode=DR,
                                    )
                                nc.vector._custom_dve(
                                    NOISY_OP, out=b2_t[:, o, sg],
                                    in0=p2[:, :], in1=a2_t[n1][:, o, :],
                                    s0=0.25, s1=32.0)

    nc.compile()
    return nc


_NC_CACHE: dict[int, object] = {}


def _get_nc(repeat: int = 1):
    if repeat not in _NC_CACHE:
        _NC_CACHE[repeat] = build_nc(repeat)
    return _NC_CACHE[repeat]


def make_in_maps(x, u2, u3, W1, W2, W3, W4, **_unused):
    """Host preprocessing -> per-core input dicts."""
    fp8_np = mybir.dt.np(FP8)

    x = np.ascontiguousarray(np.asarray(x, dtype=np.float32))
    W1b = np.sign(np.asarray(W1, dtype=np.float32))
    # mean(h1, axis=0) = sign(W1) @ mean(x, axis=0), in float64; negated and
    # pre-scaled so the device computes sign(2^21 h1 + bias).
    mu1 = (W1b.astype(np.float64) @ x.mean(axis=0, dtype=np.float64)).astype(
        np.float32)
    c1 = np.ascontiguousarray(
        (np.float32(-XSCALE) * mu1).reshape(OC, 128).T)  # [128, OC]

    # balanced base-32 fixed-point split: round(x*2^21) = sum d_j 32^(4-j),
    # digits d_j in [-16, 16] - every one exact in fp8e4m3.
    assert np.abs(x).max() * XSCALE < 16.5 * (32 ** 5 - 1) / 31, "x overflow"
    n = np.rint(x.T.astype(np.float64) * XSCALE).astype(np.int64)  # [784, B]
    digs = []
    for _ in range(5):
        d = ((n + 16) % 32) - 16
        n = (n - d) >> 5
        digs.append(d)                      # LSB first: d4, d3, d2, d1, d0
    assert np.all(n == 0), "digit overflow"
    d4, d3, d2, d1, d0 = digs

    # xt slice layout [128, KXT, SW]: A stream (d0,d1,d2 = 2352 rows, pad
    # to KA*128) then B stream (d3,d4 = 1568 rows, pad to KB*128).
    xt_all = np.zeros((KXT * 128, B), dtype=fp8_np)
    for j, d in enumerate((d0, d1, d2)):
        xt_all[j * D_IN:(j + 1) * D_IN] = d.astype(np.float32).astype(fp8_np)
    for j, d in enumerate((d3, d4)):
        r = KA * 128 + j * D_IN
        xt_all[r:r + D_IN] = d.astype(np.float32).astype(fp8_np)

    # stored weight stream per out-chunk: [32*w, w, w/32] (2352 rows + pad),
    # shared by groups A and B (B reads the 2/3 prefix).
    w1p = np.zeros((KA * 128, D_H), dtype=fp8_np)
    for j, sc in enumerate((32.0, 1.0, 1.0 / 32.0)):
        w1p[j * D_IN:(j + 1) * D_IN] = (
            W1b.T * np.float32(sc)).astype(fp8_np)
    # [o][p][k][m]: one contiguous DMA per 128-feature output block.
    w1_blocks = np.ascontiguousarray(
        w1p.reshape(KA, 128, OC, 128).transpose(2, 1, 0, 3))

    pt = _prob_table()
    t2tab = _t2_table(fp8_np)
    a2f = t2tab[(_flip_thresholds(np.asarray(u2), pt) // 2)]   # [B, 1024]
    a3f = t2tab[(_flip_thresholds(np.asarray(u3), pt) // 2)]

    def _hidden_w(w, sc):
        wt = (np.sign(np.asarray(w, np.float32)) * np.float32(sc)
              ).T.astype(fp8_np)                               # [K, M]
        return np.ascontiguousarray(
            wt.reshape(KH, 128, wt.shape[1]).transpose(1, 0, 2))

    w2t = _hidden_w(W2, 0.5)               # [128, 8, 1024], +-0.5
    w3t = _hidden_w(W3, 0.5)
    w4t = _hidden_w(W4, 1.0)               # [128, 8, 10]
    w4p = np.zeros((128, KH, D_PAD4), dtype=fp8_np)
    w4p[:, :, :D_OUT] = w4t
    w4t = w4p

    in_maps = []
    for c in range(N_CORES):
        sl = slice(c * BC, (c + 1) * BC)
        m = {"w2": w2t, "w3": w3t, "w4": w4t, "c1": c1}
        xc = xt_all[:, sl].reshape(KXT, 128, BC)  # [k, p, col]
        for nn in range(NT):
            cs = slice(OFFS[nn], OFFS[nn] + SL[nn])
            m[f"xt{nn}"] = np.ascontiguousarray(
                xc[:, :, cs].transpose(1, 0, 2))
        for o in range(OC):
            m[f"w1_{o}"] = w1_blocks[o]
        for nm, tab in (("a2", a2f), ("a3", a3f)):
            rc = tab.T[:, sl].astype(fp8_np).reshape(OC, 128, BC)
            for nn in range(NT):
                cs = slice(OFFS[nn], OFFS[nn] + SL[nn])
                m[f"{nm}_{nn}"] = np.ascontiguousarray(
                    rc[:, :, cs].transpose(1, 0, 2))
        in_maps.append(m)
    return in_maps


def kernel(x, u2, u3, W1, W2, W3, W4,
           g1=None, b1=None, g2=None, b2=None, g3=None, b3=None):
    for g in (g1, g2, g3):
        assert g is None or np.all(np.asarray(g) > 0), "kernel assumes g > 0"
    for b in (b1, b2, b3):
        assert b is None or np.all(np.asarray(b) == 0), "kernel assumes b == 0"

    nc = _get_nc(repeat=1)
    in_maps = make_in_maps(x, u2, u3, W1, W2, W3, W4)
    res = run_bass_kernel_spmd(nc, in_maps, core_ids=list(range(N_CORES)))

    out = np.empty((B, D_OUT), dtype=np.float32)
    for c in range(N_CORES):
        out[c * BC:(c + 1) * BC, :] = res.results[c]["out"].T
    return out


# revision 32
# speedup vs baseline: 1.0478x; 1.0153x over previous
"""Bass/Trainium2 kernel for a binarized NN (BNN) forward pass, data-parallel
over 8 NeuronCores.

Reference semantics (fp32):
    h1 = x @ sign(W1).T;  b1 = sign(h1 - mean(h1, axis=0))        # g=1, b=0
    h2 = b1 @ sign(W2).T; b2 = noisy_sign(h2, u2)                  # BN+sign is
    h3 = b2 @ sign(W3).T; b3 = noisy_sign(h3, u3)                  # identity on +-1
    out = b3 @ sign(W4).T

Implementation summary (all arithmetic exact small-integer, as in the
fixed-point analysis below):
  * Layer 1 contracts a balanced base-32 fixed-point split of x:
    round(x*2^21) = sum_j d_j 32^(4-j) with digits d_j in [-16,16], every
    one exact in fp8e4m3.  Two PSUM groups share one stored weight stream
    [32*w, w, w/32] (w = sign(W1)): group A moves digits (d0,d1,d2) over
    the full stream, group B moves (d3,d4) over its 2/3 prefix, so
    h1*2^21 = 2^15*A + B.  All products and DoubleRow pair-sums are
    integers scaled by powers of two within the pair-adder's ~12-bit
    mantissa, and each group's PSUM total stays under 2^24/granularity,
    so accumulation is exact.  17 DR pair-matmuls per output chunk
    (vs 20 for the 6-nibble scheme).
  * ACT stages bsb = 2^15*A + c1 (c1 = -2^21*mean(h1), computed on host in
    float64); a single custom DVE op then emits
    b1 = clip((B + bsb) * 2^35, -1, 1) = sign(h1 - mu1) directly.  The
    only inexactness is one fp32 rounding in the stage/add (~2^-19 of h1)
    plus the 2^-22 input quantization - both far below the reference's
    own matmul rounding scale.
  * W2/W3 ship as +-0.5 so PSUM holds h' = h/2 (an exact integer <= 512).
    The stochastic flip (u < 0.5 exp(-h^2/50), |h| <= 50) depends only on
    A(u) = smallest even a with p(a) <= u: flip <=> |h| < A.  With
    t = h' - 1/4 and T2 ~ ((A-1)/2)^2 (fp8-rounded inside its decision
    margin, -1 when A = 0),
        noisy = clip(32 * t * (t*t - T2), -1, 1)
    equals the exact noisy sign for every integer h, so the u-derived
    tables ship as fp8, halving their HBM traffic vs bf16.
  * b in {+-1} and sign(W4) in {+-1} make the output exact integers.

Layout is feature-major: activations live as [features(partitions),
batch(free)].  Batch 16384 is sharded 2048/core; each core pipelines four
512-column slices through all four layers, skewed one slice per layer.
Each iteration has an A-phase (8 L1 group-A chains + ACT stages, no DVE
dependencies - DVE backlog from the previous iteration drains under it)
and a B-phase (L1 group-B chains + SignAdd, interleaved with the previous
slices' L2/L3/L4 chains and noisy-sign ops).
"""

from contextlib import ExitStack

import numpy as np

import concourse.bass as bass  # noqa: F401
import concourse.tile as tile
from concourse import bacc, mybir
from concourse.bass_utils import run_bass_kernel_spmd

F32 = mybir.dt.float32
BF16 = mybir.dt.bfloat16
FP8 = mybir.dt.float8e4
ACTF = mybir.ActivationFunctionType
DR = mybir.MatmulPerfMode.DoubleRow

N_CORES = 8
B = 16384                 # full batch
BC = B // N_CORES         # batch per core
D_IN = 784                # layer-1 input features
D_H = 1024                # hidden features
D_OUT = 10                # output features
D_PAD4 = 16               # L4 stationary dim padded for DoubleRow
XSCALE = float(2 ** 21)   # fixed-point scale of |x|
ASCALE = float(2 ** 15)   # h1*2^21 = 2^15*A + B
KH = D_H // 128           # 8 k-chunks for hidden layers
OC = D_H // 128           # 8 output-feature chunks
# Layer-1 chunk geometry: A stream = digits (d0,d1,d2) = 2352 rows ->
# 10 DR pair-calls (20 chunks padded); B stream = (d3,d4) = 1568 rows ->
# 7 pair-calls (14 chunks padded).
KA = 20                   # A-stream chunks (incl pad), 10 pair-calls
KB = 14                   # B-stream chunks (incl pad), 7 pair-calls
KXT = KA + KB             # chunks per xt slice
# Batch-column slice widths: 512 amortizes per-op overheads and keeps DMA
# runs >= 512B; the last two slices narrow to 256 to shrink the DVE-bound
# pipeline drain (the tail's noisy-sign ops can't hide under L1 PE work).
SL = [512, 512, 512, 256, 256]
OFFS = [sum(SL[:i]) for i in range(len(SL))]
NT = len(SL)
assert sum(SL) == BC

# float32(0.5*exp(-(a*a)/50)) for a = 0,2,...,50 (bit-exact fallback table).
_PTABLE_BITS = [
    0x3F000000, 0x3EEC515A, 0x3EB9E4E3, 0x3E79375C, 0x3E0E5ACB, 0x3D8A9501,
    0x3CE5ED93, 0x3C2289CB, 0x3B43D285, 0x3A4909DD, 0x392FE09E, 0x38031DFC,
    0x36A696B8, 0x35345CD8, 0x33A6674D, 0x3202D2C5, 0x302F4A31, 0x2E4824C7,
    0x2C42BB52, 0x2A2173E9, 0x27E4229E, 0x258959AD, 0x230CEE5E, 0x207672F6,
    0x1DB79FE2, 0x1AE92B5E,
]


def _prob_table() -> np.ndarray:
    """p(a) for a = 0,2,...,50, bit-matching the reference's jnp.exp."""
    try:
        import jax.numpy as jnp

        a = np.arange(0, 51, 2, dtype=np.float32)
        p = np.asarray(0.5 * jnp.exp(-(jnp.asarray(a) * a) / (2.0 * 5.0**2)),
                       dtype=np.float32)
        if p.shape == (26,) and np.all(np.diff(p) < 0):
            return p
    except Exception:
        pass
    return np.array(_PTABLE_BITS, dtype=np.uint32).view(np.float32)


def _flip_thresholds(u: np.ndarray, ptable: np.ndarray) -> np.ndarray:
    """A(u): flip <=> |h| < A. A = 52 - 2 * #{a : p(a) <= u}."""
    tab = ptable[::-1].copy()  # ascending: p(50), p(48), ..., p(0)
    idx = np.searchsorted(tab, u, side="right")
    return (52 - 2 * idx).astype(np.int64)


def _t2_entry(A: int, fp8_np) -> float:
    """fp8 threshold T2 for even A: separates t^2 = (h'-1/4)^2 at integer
    h' into flip (|h'| < A/2) vs keep.  Must lie strictly inside
    ((A/2-0.75)^2, (A/2-0.25)^2) after fp8 rounding, with s1=32 margins."""
    if A == 0:
        return -1.0
    lo, hi = (A / 2 - 0.75) ** 2, (A / 2 - 0.25) ** 2
    cand = float(np.asarray((A / 2 - 0.5) ** 2, np.float32).astype(fp8_np))
    if not (lo < cand < hi):
        # scan fp8-representable values inside the interval
        for v in np.linspace(lo, hi, 64)[1:-1]:
            c = float(np.asarray(v, np.float32).astype(fp8_np))
            if lo < c < hi:
                cand = c
                break
        else:
            raise AssertionError(f"no fp8 threshold for A={A}")
    return cand


def _t2_table(fp8_np) -> np.ndarray:
    """T2 per A-index (A = 0,2,...,52), fp8 values as fp32, capped at 448."""
    out = np.empty(27, np.float32)
    for i in range(27):
        A = 2 * i
        if A >= 34:
            # fp8e4m3 (IEEE, max 240) can't hold larger thresholds.
            # P(A >= 34) = 6.3e-10/element: ~0.02 occurrences expected in
            # the whole problem, and an error additionally needs |h|>=32.
            out[i] = 240.0
        else:
            out[i] = _t2_entry(A, fp8_np)
    # margin check: |32*t*(t^2-T2)| >= 1 with correct sign, h' integer
    hp = np.arange(-600, 601)
    t = hp - 0.25
    for i in range(27):
        A = 2 * i
        w = 32.0 * t * (t * t - out[i])
        assert np.all(np.abs(w) >= 1.0), (A, np.abs(w).min())
        if A >= 34:
            continue
        flip = np.abs(2 * hp) < A
        s = np.where(hp > 0, 1.0, -1.0)
        want = np.where(flip, -s, s)
        assert np.all(np.clip(w, -1.0, 1.0) == want), (A,)
    return out


# ---------------------------------------------------------------------------
# Custom fused DVE ops.
#   NOISY:   out = clip(s1 * (in0-s0) * ((in0-s0)^2 - in1), -1, 1)
#   SIGNADD: out = clip((in0 + in1) * s1, -1, 1)
# ---------------------------------------------------------------------------

_NOISY_OP_NAME = "NOISY_SIGN_PM1_ANT"
_SIGNADD_OP_NAME = "SIGN_ADD_PM1_ANT"


def _noisy_ref(in0, in1, c0, c1, c2):
    t = np.asarray(in0, np.float32) - np.float32(c0)
    r2 = np.asarray(in1, np.float32).reshape(t.shape)
    w = (t * (t * t - r2)) * np.float32(c1)
    return np.maximum(np.minimum(w, np.float32(1.0)), np.float32(-1.0))


def _signadd_ref(in0, in1, c0, c1, c2):
    v = (np.asarray(in0, np.float32)
         + np.asarray(in1, np.float32).reshape(np.shape(in0)))
    w = v * np.float32(c1)
    return np.maximum(np.minimum(w, np.float32(1.0)), np.float32(-1.0))


def _register_op(name, body_fn, ref):
    from concourse import dve_ops
    from concourse.dve_spec import (C0, C1, One, Spec, Src0, Src1, Zero,
                                    lower, maxx, minn)
    from concourse.dve_uop import DveOpSpec

    for op in dve_ops.OPS:
        if op.name == name:
            return op

    body = body_fn(C0, C1, One, Zero, Src0, Src1, maxx, minn)
    spec = Spec(body=body, reference=ref)

    row = dve_ops._CUSTOM_DVE_ROW_BASE + len(dve_ops.OPS)
    assert row < 0x20, "custom-DVE opcode rows exhausted"
    shas = {}
    for ver in ("v3", "v4"):
        d = DveOpSpec(name=name, opcode=row, uops=lower(spec, ver=ver),
                      rd1_en=True)
        shas[ver] = d.sha(ver)
    op = dve_ops.DveOp(name, spec, subdim=False, uops_sha=shas)
    dve_ops.OPS.append(op)
    dve_ops.CUSTOM_DVE_SPECS[name] = spec
    dve_ops._SUB_OPCODE_FOR_NAME[name] = row
    return op


def _noisy_body(C0, C1, One, Zero, Src0, Src1, maxx, minn):
    t = Src0 - C0
    w = (t * ((t * t) - Src1)) * C1
    return maxx(minn(w, One), Zero - One)


def _signadd_body(C0, C1, One, Zero, Src0, Src1, maxx, minn):
    w = (Src0 + Src1) * C1
    return maxx(minn(w, One), Zero - One)


NOISY_OP = _register_op(_NOISY_OP_NAME, _noisy_body, _noisy_ref)
SIGNADD_OP = _register_op(_SIGNADD_OP_NAME, _signadd_body, _signadd_ref)


def build_nc(repeat: int = 1):
    """Build the per-core Bass program (same program on all 8 cores)."""
    nc = bacc.Bacc("TRN2", target_bir_lowering=False, debug=False,
                   num_devices=N_CORES)

    xt = [nc.dram_tensor(f"xt{n}", [128, KXT, SL[n]], FP8,
                         kind="ExternalInput").ap() for n in range(NT)]
    w1 = [nc.dram_tensor(f"w1_{o}", [128, KA, 128], FP8,
                         kind="ExternalInput").ap() for o in range(OC)]
    a2 = [nc.dram_tensor(f"a2_{n}", [128, OC, SL[n]], FP8,
                         kind="ExternalInput").ap() for n in range(NT)]
    a3 = [nc.dram_tensor(f"a3_{n}", [128, OC, SL[n]], FP8,
                         kind="ExternalInput").ap() for n in range(NT)]
    w2 = nc.dram_tensor("w2", [128, KH, D_H], FP8, kind="ExternalInput").ap()
    w3 = nc.dram_tensor("w3", [128, KH, D_H], FP8, kind="ExternalInput").ap()
    w4 = nc.dram_tensor("w4", [128, KH, D_PAD4], FP8,
                        kind="ExternalInput").ap()
    c1 = nc.dram_tensor("c1", [128, OC], F32, kind="ExternalInput").ap()
    out = nc.dram_tensor("out", [D_OUT, BC], F32, kind="ExternalOutput").ap()

    with tile.TileContext(nc) as tc:
        with ExitStack() as ctx:
            consts = ctx.enter_context(tc.tile_pool(name="consts", bufs=1))
            panels = ctx.enter_context(tc.tile_pool(name="panels", bufs=1))
            bsb0 = ctx.enter_context(tc.tile_pool(name="bsb0", bufs=1))
            xtp = ctx.enter_context(tc.tile_pool(name="xtp", bufs=2))
            apool = ctx.enter_context(tc.tile_pool(name="apool", bufs=4))
            opool = ctx.enter_context(tc.tile_pool(name="opool", bufs=2))

            c1_t = consts.tile([128, OC], F32, tag="c1")
            w1_t = consts.tile([128, OC * KA, 128], FP8, tag="w1")
            w2_t = consts.tile([128, KH, D_H], FP8, tag="w2")
            w3_t = consts.tile([128, KH, D_H], FP8, tag="w3")
            w4_t = consts.tile([128, KH, D_PAD4], FP8, tag="w4")

            # +-1 activation panels, feature-major fp8.
            b1_t = panels.tile([128, KH, BC], FP8, tag="b1")
            b2_t = panels.tile([128, KH, BC], FP8, tag="b2")
            b3_t = panels.tile([128, KH, BC], FP8, tag="b3")

            # Priority order on the shared DMA engine: enough w1/xt to start
            # and continuously feed L1's A-phase, with the B-stream (xt
            # chunks KA..) and later consts behind it.
            # Front DMA, in deadline order on the shared DMA engine: c1
            # (stage(0) needs it early), then w1 blocks and xt0 A-chunk
            # pairs interleaved to track the PE's consumption rate, then the
            # B-stream and the remaining w1 blocks.
            xt_t: dict[int, object] = {}
            xt_t[0] = xtp.tile([128, KXT, SL[0]], FP8, tag="xt",
                               name="xt_t0")
            nc.sync.dma_start(c1_t[:], c1[:, :])
            nc.sync.dma_start(w1_t[:, 0:4, :], w1[0][:, 0:4, :])
            nc.sync.dma_start(xt_t[0][:, 0:2, :], xt[0][:, 0:2, :])
            nc.sync.dma_start(xt_t[0][:, 2:4, :], xt[0][:, 2:4, :])
            nc.sync.dma_start(w1_t[:, 4:12, :], w1[0][:, 4:12, :])
            nc.sync.dma_start(xt_t[0][:, 4:6, :], xt[0][:, 4:6, :])
            nc.sync.dma_start(xt_t[0][:, 6:8, :], xt[0][:, 6:8, :])
            nc.sync.dma_start(w1_t[:, 12:KA, :], w1[0][:, 12:, :])
            nc.sync.dma_start(xt_t[0][:, 8:10, :], xt[0][:, 8:10, :])
            nc.sync.dma_start(xt_t[0][:, 10:12, :], xt[0][:, 10:12, :])
            nc.sync.dma_start(w1_t[:, KA:KA + 10, :], w1[1][:, :10, :])
            nc.sync.dma_start(xt_t[0][:, 12:14, :], xt[0][:, 12:14, :])
            nc.sync.dma_start(w1_t[:, KA + 10:2 * KA, :], w1[1][:, 10:, :])
            nc.sync.dma_start(xt_t[0][:, 14:16, :], xt[0][:, 14:16, :])
            nc.sync.dma_start(w1_t[:, 2 * KA:2 * KA + 10, :],
                              w1[2][:, :10, :])
            nc.sync.dma_start(xt_t[0][:, 16:18, :], xt[0][:, 16:18, :])
            nc.sync.dma_start(w1_t[:, 2 * KA + 10:3 * KA, :],
                              w1[2][:, 10:, :])
            nc.sync.dma_start(xt_t[0][:, 18:KA, :], xt[0][:, 18:KA, :])
            nc.sync.dma_start(w1_t[:, 3 * KA:4 * KA, :], w1[3])
            nc.sync.dma_start(xt_t[0][:, KA:KA + 6, :], xt[0][:, KA:KA + 6, :])
            nc.sync.dma_start(w1_t[:, 4 * KA:5 * KA, :], w1[4])
            nc.sync.dma_start(xt_t[0][:, KA + 6:KA + 10, :],
                              xt[0][:, KA + 6:KA + 10, :])
            nc.sync.dma_start(w1_t[:, 5 * KA:6 * KA, :], w1[5])
            nc.sync.dma_start(xt_t[0][:, KA + 10:, :], xt[0][:, KA + 10:, :])
            nc.sync.dma_start(w1_t[:, 6 * KA:7 * KA, :], w1[6])
            nc.sync.dma_start(w1_t[:, 7 * KA:8 * KA, :], w1[7])
            nc.sync.dma_start(w4_t[:], w4[:, :, :])

            for _rep in range(repeat):
                with ExitStack() as rep_ctx:
                    l1a = rep_ctx.enter_context(
                        tc.tile_pool(name="l1a", bufs=2, space="PSUM"))
                    l1b = rep_ctx.enter_context(
                        tc.tile_pool(name="l1b", bufs=2, space="PSUM"))
                    l23 = rep_ctx.enter_context(
                        tc.tile_pool(name="l23", bufs=3, space="PSUM"))
                    l4ps = rep_ctx.enter_context(
                        tc.tile_pool(name="l4ps", bufs=1, space="PSUM"))

                    a2_t: dict[int, object] = {}
                    a3_t: dict[int, object] = {}
                    l4_pending = None  # (psum tile, slice index) across iters

                    def l4_finish(l4_fin):
                        """Last L4 DR pair + PSUM copy + output store."""
                        ps4, n4 = l4_fin
                        w4s = SL[n4]
                        s4 = slice(OFFS[n4], OFFS[n4] + w4s)
                        nc.tensor.matmul(ps4[:, :], w4_t[:, KH - 2:KH, :],
                                         b3_t[:, KH - 2:KH, s4],
                                         start=False, stop=True, perf_mode=DR)
                        ot = opool.tile([D_OUT, w4s], F32, tag="ot")
                        nc.scalar.activation(ot[:, :], ps4[:D_OUT, :],
                                             ACTF.Copy)
                        nc.sync.dma_start(out[:, s4], ot[:, :])

                    # Software pipeline, skewed one slice per layer.  Each
                    # iteration i runs 8 uniform rounds (one per out-chunk):
                    #   A(i,o) -> stage -> B(i,o-1) -> Sign | L2(i-1,o) +
                    #   noisy2 | L3(i-2,o) + noisy3 | L4(i-2) pair at odd o
                    # The stage writes 2^15*A + c1 INTO the group-B PSUM
                    # bank; the B pair-matmuls then accumulate on top
                    # (start=False), so b1 = Sign(PSUM) is a plain ACT op
                    # and the DVE runs only the two noisy-sign ops per
                    # round (~1.3us DVE vs ~2.7us PE: always PE-bound).
                    # B trails A by one round so the stage (ACT) hides
                    # under A(o+1).  L4(i-2)'s last pair + PSUM copy +
                    # store run in the next iteration's round 4.
                    for i in range(NT + 3):
                        l4_fin, l4_pending = l4_pending, None

                        # --- DMA prefetch for this iteration ---
                        if i + 1 < NT:
                            xt_t[i + 1] = xtp.tile([128, KXT, SL[i + 1]],
                                                   FP8, tag="xt",
                                                   name=f"xt_t{i + 1}")
                            # in pieces: the next iteration's first rounds
                            # can start on the A-stream prefix while the
                            # rest is still in flight
                            nc.sync.dma_start(xt_t[i + 1][:, :10, :],
                                              xt[i + 1][:, :10, :])
                            nc.sync.dma_start(xt_t[i + 1][:, 10:KA, :],
                                              xt[i + 1][:, 10:KA, :])
                            nc.sync.dma_start(xt_t[i + 1][:, KA:, :],
                                              xt[i + 1][:, KA:, :])
                        if i == 0:
                            nc.sync.dma_start(w2_t[:], w2[:, :, :])
                        if i == 1:
                            nc.sync.dma_start(w3_t[:], w3[:, :, :])
                        if i < NT:
                            t_a2 = apool.tile([128, OC, SL[i]], FP8,
                                              tag="a2")
                            nc.sync.dma_start(t_a2[:], a2[i])
                            a2_t[i] = t_a2
                        if 1 <= i <= NT:
                            t_a3 = apool.tile([128, OC, SL[i - 1]], FP8,
                                              tag="a3")
                            nc.sync.dma_start(t_a3[:], a3[i - 1])
                            a3_t[i - 1] = t_a3

                        n1, n2, n3 = i, i - 1, i - 2
                        if n1 < NT:
                            ws1 = SL[n1]
                            s1 = slice(OFFS[n1], OFFS[n1] + ws1)
                        if 0 <= n2 < NT:
                            ws2 = SL[n2]
                            s2 = slice(OFFS[n2], OFFS[n2] + ws2)
                        if 0 <= n3 < NT:
                            ws3 = SL[n3]
                            s3 = slice(OFFS[n3], OFFS[n3] + ws3)

                        # Iteration 0 is DMA-bound: the front-loaded w1/xt0
                        # stream is consumed 1:1 as it lands, and the merged
                        # rounds below would need the B-stream ~7us earlier
                        # than the DMA can deliver it.  So slice 0 runs the
                        # two-phase order instead: all A-chains (stage to an
                        # SBUF tile), then all B-chains with a fused DVE
                        # sign-add, giving the B-stream DMA until the
                        # A-phase's end to arrive.
                        if i == 0:
                            bsa = bsb0.tile([128, OC, ws1], F32, tag="bsb")
                            for o in range(OC):
                                pa = l1a.tile([128, ws1], F32, tag="mma")
                                for kp in range(KA // 2):
                                    nc.tensor.matmul(
                                        pa[:, :],
                                        w1_t[:, o * KA + 2 * kp:
                                             o * KA + 2 * kp + 2, :],
                                        xt_t[0][:, 2 * kp:2 * kp + 2, :],
                                        start=(kp == 0),
                                        stop=(kp == KA // 2 - 1),
                                        perf_mode=DR,
                                    )
                                nc.scalar.activation(
                                    bsa[:, o, :], pa[:, :], ACTF.Identity,
                                    scale=ASCALE, bias=c1_t[:, o:o + 1])
                            for o in range(OC):
                                pb = l1b.tile([128, ws1], F32, tag="mmb")
                                for kp in range(KB // 2):
                                    nc.tensor.matmul(
                                        pb[:, :],
                                        w1_t[:, o * KA + 2 * kp:
                                             o * KA + 2 * kp + 2, :],
                                        xt_t[0][:, KA + 2 * kp:
                                                 KA + 2 * kp + 2, :],
                                        start=(kp == 0),
                                        stop=(kp == KB // 2 - 1),
                                        perf_mode=DR,
                                    )
                                # b1 = clip((B + bsb)*2^35) = sign(h1-mu1)
                                nc.vector._custom_dve(
                                    SIGNADD_OP, out=b1_t[:, o, s1],
                                    in0=pb[:, :], in1=bsa[:, o, :],
                                    s0=0.0, s1=float(2.0 ** 35))
                            continue

                        def l1_b_chain(o, pb):
                            """Group-B pairs accumulate onto the staged
                            PSUM bank (start=False), then b1 = Sign(PSUM)."""
                            for kp in range(KB // 2):
                                # start=False on an ACT-initialized bank is
                                # intentional: accumulate onto 2^15*A + c1.
                                nc.tensor.matmul(
                                    pb[:, :],
                                    w1_t[:, o * KA + 2 * kp:
                                         o * KA + 2 * kp + 2, :],
                                    xt_t[n1][:, KA + 2 * kp:
                                             KA + 2 * kp + 2, :],
                                    start=False,
                                    stop=(kp == KB // 2 - 1),
                                    perf_mode=DR,
                                    skip_group_check=True,
                                )
                            nc.scalar.activation(
                                b1_t[:, o, s1], pb[:, :], ACTF.Sign)

                        pb_prev = None  # (o, staged PSUM tile) from last round
                        for o in range(OC):
                            if n1 < NT:
                                pa = l1a.tile([128, ws1], F32, tag="mma")
                                for kp in range(KA // 2):
                                    nc.tensor.matmul(
                                        pa[:, :],
                                        w1_t[:, o * KA + 2 * kp:
                                             o * KA + 2 * kp + 2, :],
                                        xt_t[n1][:, 2 * kp:2 * kp + 2, :],
                                        start=(kp == 0),
                                        stop=(kp == KA // 2 - 1),
                                        perf_mode=DR,
                                    )
                                # PSUM init for group B: 2^15*A + c1
                                pb = l1b.tile([128, ws1], F32, tag="mmb")
                                nc.scalar.activation(
                                    pb[:, :], pa[:, :], ACTF.Identity,
                                    scale=ASCALE, bias=c1_t[:, o:o + 1])
                                if pb_prev is not None:
                                    l1_b_chain(*pb_prev)
                                pb_prev = (o, pb)
                            if o == 3 and l4_fin is not None:
                                l4_finish(l4_fin)
                                l4_fin = None
                            if 0 <= n2 < NT - 1:
                                p2 = l23.tile([128, ws2], F32, tag="mm23",
                                              name="p2")
                                for kp in range(KH // 2):
                                    nc.tensor.matmul(
                                        p2[:, :],
                                        w2_t[:, 2 * kp:2 * kp + 2,
                                             o * 128:(o + 1) * 128],
                                        b1_t[:, 2 * kp:2 * kp + 2, s2],
                                        start=(kp == 0),
                                        stop=(kp == KH // 2 - 1),
                                        perf_mode=DR,
                                    )
                                nc.vector._custom_dve(
                                    NOISY_OP, out=b2_t[:, o, s2],
                                    in0=p2[:, :], in1=a2_t[n2][:, o, :],
                                    s0=0.25, s1=32.0)
                            if 0 <= n3 < NT:
                                p3 = l23.tile([128, ws3], F32, tag="mm23",
                                              name="p3")
                                for kp in range(KH // 2):
                                    nc.tensor.matmul(
                                        p3[:, :],
                                        w3_t[:, 2 * kp:2 * kp + 2,
                                             o * 128:(o + 1) * 128],
                                        b2_t[:, 2 * kp:2 * kp + 2, s3],
                                        start=(kp == 0),
                                        stop=(kp == KH // 2 - 1),
                                        perf_mode=DR,
                                    )
                                nc.vector._custom_dve(
                                    NOISY_OP, out=b3_t[:, o, s3],
                                    in0=p3[:, :], in1=a3_t[n3][:, o, :],
                                    s0=0.25, s1=32.0)
                                if o >= 3 and o % 2 == 1:
                                    kp = (o - 3) // 2
                                    if kp == 0:
                                        ps4 = l4ps.tile([D_PAD4, ws3], F32,
                                                        tag="mm4")
                                        l4_pending = (ps4, n3)
                                    nc.tensor.matmul(
                                        ps4[:, :],
                                        w4_t[:, 2 * kp:2 * kp + 2, :],
                                        b3_t[:, 2 * kp:2 * kp + 2, s3],
                                        start=(kp == 0), stop=False,
                                        perf_mode=DR)
                        if pb_prev is not None:
                            l1_b_chain(*pb_prev)
                        if l4_fin is not None:  # iterations with empty loop
                            l4_finish(l4_fin)

                        # --- gamma-phase (last L1 iteration only): run the
                        # final slice's L2 immediately, so its noisy-sign DVE
                        # work overlaps the remaining tail PE work instead of
                        # serializing after it.
                        if i == NT - 1:
                            sg = s1
                            for o in range(OC):
                                p2 = l23.tile([128, ws1], F32, tag="mm23",
                                              name="p2g")
                                for kp in range(KH // 2):
                                    nc.tensor.matmul(
                                        p2[:, :],
                                        w2_t[:, 2 * kp:2 * kp + 2,
                                             o * 128:(o + 1) * 128],
                                        b1_t[:, 2 * kp:2 * kp + 2, sg],
                                        start=(kp == 0),
                                        stop=(kp == KH // 2 - 1),
                                        perf_mode=DR,
                                    )
                                nc.vector._custom_dve(
                                    NOISY_OP, out=b2_t[:, o, sg],
                                    in0=p2[:, :], in1=a2_t[n1][:, o, :],
                                    s0=0.25, s1=32.0)

    nc.compile()
    return nc


_NC_CACHE: dict[int, object] = {}


def _get_nc(repeat: int = 1):
    if repeat not in _NC_CACHE:
        _NC_CACHE[repeat] = build_nc(repeat)
    return _NC_CACHE[repeat]


def make_in_maps(x, u2, u3, W1, W2, W3, W4, **_unused):
    """Host preprocessing -> per-core input dicts."""
    fp8_np = mybir.dt.np(FP8)

    x = np.ascontiguousarray(np.asarray(x, dtype=np.float32))
    W1b = np.sign(np.asarray(W1, dtype=np.float32))
    # mean(h1, axis=0) = sign(W1) @ mean(x, axis=0), in float64; negated and
    # pre-scaled so the device computes sign(2^21 h1 + bias).
    mu1 = (W1b.astype(np.float64) @ x.mean(axis=0, dtype=np.float64)).astype(
        np.float32)
    c1 = np.ascontiguousarray(
        (np.float32(-XSCALE) * mu1).reshape(OC, 128).T)  # [128, OC]

    # balanced base-32 fixed-point split: round(x*2^21) = sum d_j 32^(4-j),
    # digits d_j in [-16, 16] - every one exact in fp8e4m3.
    assert np.abs(x).max() * XSCALE < 16.5 * (32 ** 5 - 1) / 31, "x overflow"
    n = np.rint(x.T.astype(np.float64) * XSCALE).astype(np.int64)  # [784, B]
    digs = []
    for _ in range(5):
        d = ((n + 16) % 32) - 16
        n = (n - d) >> 5
        digs.append(d)                      # LSB first: d4, d3, d2, d1, d0
    assert np.all(n == 0), "digit overflow"
    d4, d3, d2, d1, d0 = digs

    # xt slice layout [128, KXT, SW]: A stream (d0,d1,d2 = 2352 rows, pad
    # to KA*128) then B stream (d3,d4 = 1568 rows, pad to KB*128).
    xt_all = np.zeros((KXT * 128, B), dtype=fp8_np)
    for j, d in enumerate((d0, d1, d2)):
        xt_all[j * D_IN:(j + 1) * D_IN] = d.astype(np.float32).astype(fp8_np)
    for j, d in enumerate((d3, d4)):
        r = KA * 128 + j * D_IN
        xt_all[r:r + D_IN] = d.astype(np.float32).astype(fp8_np)

    # stored weight stream per out-chunk: [32*w, w, w/32] (2352 rows + pad),
    # shared by groups A and B (B reads the 2/3 prefix).
    w1p = np.zeros((KA * 128, D_H), dtype=fp8_np)
    for j, sc in enumerate((32.0, 1.0, 1.0 / 32.0)):
        w1p[j * D_IN:(j + 1) * D_IN] = (
            W1b.T * np.float32(sc)).astype(fp8_np)
    # [o][p][k][m]: one contiguous DMA per 128-feature output block.
    w1_blocks = np.ascontiguousarray(
        w1p.reshape(KA, 128, OC, 128).transpose(2, 1, 0, 3))

    pt = _prob_table()
    t2tab = _t2_table(fp8_np)
    a2f = t2tab[(_flip_thresholds(np.asarray(u2), pt) // 2)]   # [B, 1024]
    a3f = t2tab[(_flip_thresholds(np.asarray(u3), pt) // 2)]

    def _hidden_w(w, sc):
        wt = (np.sign(np.asarray(w, np.float32)) * np.float32(sc)
              ).T.astype(fp8_np)                               # [K, M]
        return np.ascontiguousarray(
            wt.reshape(KH, 128, wt.shape[1]).transpose(1, 0, 2))

    w2t = _hidden_w(W2, 0.5)               # [128, 8, 1024], +-0.5
    w3t = _hidden_w(W3, 0.5)
    w4t = _hidden_w(W4, 1.0)               # [128, 8, 10]
    w4p = np.zeros((128, KH, D_PAD4), dtype=fp8_np)
    w4p[:, :, :D_OUT] = w4t
    w4t = w4p

    in_maps = []
    for c in range(N_CORES):
        sl = slice(c * BC, (c + 1) * BC)
        m = {"w2": w2t, "w3": w3t, "w4": w4t, "c1": c1}
        xc = xt_all[:, sl].reshape(KXT, 128, BC)  # [k, p, col]
        for nn in range(NT):
            cs = slice(OFFS[nn], OFFS[nn] + SL[nn])
            m[f"xt{nn}"] = np.ascontiguousarray(
                xc[:, :, cs].transpose(1, 0, 2))
        for o in range(OC):
            m[f"w1_{o}"] = w1_blocks[o]
        for nm, tab in (("a2", a2f), ("a3", a3f)):
            rc = tab.T[:, sl].astype(fp8_np).reshape(OC, 128, BC)
            for nn in range(NT):
                cs = slice(OFFS[nn], OFFS[nn] + SL[nn])
                m[f"{nm}_{nn}"] = np.ascontiguousarray(
                    rc[:, :, cs].transpose(1, 0, 2))
        in_maps.append(m)
    return in_maps


def kernel(x, u2, u3, W1, W2, W3, W4,
           g1=None, b1=None, g2=None, b2=None, g3=None, b3=None):
    for g in (g1, g2, g3):
        assert g is None or np.all(np.asarray(g) > 0), "kernel assumes g > 0"
    for b in (b1, b2, b3):
        assert b is None or np.all(np.asarray(b) == 0), "kernel assumes b == 0"

    nc = _get_nc(repeat=1)
    in_maps = make_in_maps(x, u2, u3, W1, W2, W3, W4)
    res = run_bass_kernel_spmd(nc, in_maps, core_ids=list(range(N_CORES)))

    out = np.empty((B, D_OUT), dtype=np.float32)
    for c in range(N_CORES):
        out[c * BC:(c + 1) * BC, :] = res.results[c]["out"].T
    return out


# revision 36
# speedup vs baseline: 1.0582x; 1.0100x over previous
"""Bass/Trainium2 kernel for a binarized NN (BNN) forward pass, data-parallel
over 8 NeuronCores.

Reference semantics (fp32):
    h1 = x @ sign(W1).T;  b1 = sign(h1 - mean(h1, axis=0))        # g=1, b=0
    h2 = b1 @ sign(W2).T; b2 = noisy_sign(h2, u2)                  # BN+sign is
    h3 = b2 @ sign(W3).T; b3 = noisy_sign(h3, u3)                  # identity on +-1
    out = b3 @ sign(W4).T

Implementation summary (all arithmetic exact small-integer, as in the
fixed-point analysis below):
  * Layer 1 contracts a balanced base-32 fixed-point split of x:
    round(x*2^21) = sum_j d_j 32^(4-j) with digits d_j in [-16,16], every
    one exact in fp8e4m3.  Two PSUM groups share one stored weight stream
    [32*w, w, w/32] (w = sign(W1)): group A moves digits (d0,d1,d2) over
    the full stream, group B moves (d3,d4) over its 2/3 prefix, so
    h1*2^21 = 2^15*A + B.  All products and DoubleRow pair-sums are
    integers scaled by powers of two within the pair-adder's ~12-bit
    mantissa, and each group's PSUM total stays under 2^24/granularity,
    so accumulation is exact.  17 DR pair-matmuls per output chunk
    (vs 20 for the 6-nibble scheme).
  * ACT stages bsb = 2^15*A + c1 (c1 = -2^21*mean(h1), computed on host in
    float64); a single custom DVE op then emits
    b1 = clip((B + bsb) * 2^35, -1, 1) = sign(h1 - mu1) directly.  The
    only inexactness is one fp32 rounding in the stage/add (~2^-19 of h1)
    plus the 2^-22 input quantization - both far below the reference's
    own matmul rounding scale.
  * W2/W3 ship as +-0.5 so PSUM holds h' = h/2 (an exact integer <= 512).
    The stochastic flip (u < 0.5 exp(-h^2/50), |h| <= 50) depends only on
    A(u) = smallest even a with p(a) <= u: flip <=> |h| < A.  With
    t = h' - 1/4 and T2 ~ ((A-1)/2)^2 (fp8-rounded inside its decision
    margin, -1 when A = 0),
        noisy = clip(32 * t * (t*t - T2), -1, 1)
    equals the exact noisy sign for every integer h, so the u-derived
    tables ship as fp8, halving their HBM traffic vs bf16.
  * b in {+-1} and sign(W4) in {+-1} make the output exact integers.

Layout is feature-major: activations live as [features(partitions),
batch(free)].  Batch 16384 is sharded 2048/core; each core pipelines four
512-column slices through all four layers, skewed one slice per layer.
Each iteration has an A-phase (8 L1 group-A chains + ACT stages, no DVE
dependencies - DVE backlog from the previous iteration drains under it)
and a B-phase (L1 group-B chains + SignAdd, interleaved with the previous
slices' L2/L3/L4 chains and noisy-sign ops).
"""

from contextlib import ExitStack

import numpy as np

import concourse.bass as bass  # noqa: F401
import concourse.tile as tile
from concourse import bacc, mybir
from concourse.bass_utils import run_bass_kernel_spmd

F32 = mybir.dt.float32
BF16 = mybir.dt.bfloat16
FP8 = mybir.dt.float8e4
ACTF = mybir.ActivationFunctionType
DR = mybir.MatmulPerfMode.DoubleRow

N_CORES = 8
B = 16384                 # full batch
BC = B // N_CORES         # batch per core
D_IN = 784                # layer-1 input features
D_H = 1024                # hidden features
D_OUT = 10                # output features
D_PAD4 = 16               # L4 stationary dim padded for DoubleRow
XSCALE = float(2 ** 21)   # fixed-point scale of |x|
ASCALE = float(2 ** 15)   # h1*2^21 = 2^15*A + B
KH = D_H // 128           # 8 k-chunks for hidden layers
OC = D_H // 128           # 8 output-feature chunks
# Layer-1 chunk geometry: A stream = digits (d0,d1,d2) = 2352 rows ->
# 10 DR pair-calls (20 chunks padded); B stream = (d3,d4) = 1568 rows ->
# 7 pair-calls (14 chunks padded).
KA = 20                   # A-stream chunks (incl pad), 10 pair-calls
KB = 14                   # B-stream chunks (incl pad), 7 pair-calls
KXT = KA + KB             # chunks per xt slice
# Batch-column slice widths: 512 amortizes per-op overheads and keeps DMA
# runs >= 512B; the last two slices narrow to 256 to shrink the DVE-bound
# pipeline drain (the tail's noisy-sign ops can't hide under L1 PE work).
SL = [512, 512, 512, 256, 256]
OFFS = [sum(SL[:i]) for i in range(len(SL))]
NT = len(SL)
assert sum(SL) == BC

# float32(0.5*exp(-(a*a)/50)) for a = 0,2,...,50 (bit-exact fallback table).
_PTABLE_BITS = [
    0x3F000000, 0x3EEC515A, 0x3EB9E4E3, 0x3E79375C, 0x3E0E5ACB, 0x3D8A9501,
    0x3CE5ED93, 0x3C2289CB, 0x3B43D285, 0x3A4909DD, 0x392FE09E, 0x38031DFC,
    0x36A696B8, 0x35345CD8, 0x33A6674D, 0x3202D2C5, 0x302F4A31, 0x2E4824C7,
    0x2C42BB52, 0x2A2173E9, 0x27E4229E, 0x258959AD, 0x230CEE5E, 0x207672F6,
    0x1DB79FE2, 0x1AE92B5E,
]


def _prob_table() -> np.ndarray:
    """p(a) for a = 0,2,...,50, bit-matching the reference's jnp.exp."""
    try:
        import jax.numpy as jnp

        a = np.arange(0, 51, 2, dtype=np.float32)
        p = np.asarray(0.5 * jnp.exp(-(jnp.asarray(a) * a) / (2.0 * 5.0**2)),
                       dtype=np.float32)
        if p.shape == (26,) and np.all(np.diff(p) < 0):
            return p
    except Exception:
        pass
    return np.array(_PTABLE_BITS, dtype=np.uint32).view(np.float32)


def _flip_thresholds(u: np.ndarray, ptable: np.ndarray) -> np.ndarray:
    """A(u): flip <=> |h| < A. A = 52 - 2 * #{a : p(a) <= u}."""
    tab = ptable[::-1].copy()  # ascending: p(50), p(48), ..., p(0)
    idx = np.searchsorted(tab, u, side="right")
    return (52 - 2 * idx).astype(np.int64)


def _t2_entry(A: int, fp8_np) -> float:
    """fp8 threshold T2 for even A: separates t^2 = (h'-1/4)^2 at integer
    h' into flip (|h'| < A/2) vs keep.  Must lie strictly inside
    ((A/2-0.75)^2, (A/2-0.25)^2) after fp8 rounding, with s1=32 margins."""
    if A == 0:
        return -1.0
    lo, hi = (A / 2 - 0.75) ** 2, (A / 2 - 0.25) ** 2
    cand = float(np.asarray((A / 2 - 0.5) ** 2, np.float32).astype(fp8_np))
    if not (lo < cand < hi):
        # scan fp8-representable values inside the interval
        for v in np.linspace(lo, hi, 64)[1:-1]:
            c = float(np.asarray(v, np.float32).astype(fp8_np))
            if lo < c < hi:
                cand = c
                break
        else:
            raise AssertionError(f"no fp8 threshold for A={A}")
    return cand


def _t2_table(fp8_np) -> np.ndarray:
    """T2 per A-index (A = 0,2,...,52), fp8 values as fp32, capped at 448."""
    out = np.empty(27, np.float32)
    for i in range(27):
        A = 2 * i
        if A >= 34:
            # fp8e4m3 (IEEE, max 240) can't hold larger thresholds.
            # P(A >= 34) = 6.3e-10/element: ~0.02 occurrences expected in
            # the whole problem, and an error additionally needs |h|>=32.
            out[i] = 240.0
        else:
            out[i] = _t2_entry(A, fp8_np)
    # margin check: |32*t*(t^2-T2)| >= 1 with correct sign, h' integer
    hp = np.arange(-600, 601)
    t = hp - 0.25
    for i in range(27):
        A = 2 * i
        w = 32.0 * t * (t * t - out[i])
        assert np.all(np.abs(w) >= 1.0), (A, np.abs(w).min())
        if A >= 34:
            continue
        flip = np.abs(2 * hp) < A
        s = np.where(hp > 0, 1.0, -1.0)
        want = np.where(flip, -s, s)
        assert np.all(np.clip(w, -1.0, 1.0) == want), (A,)
    return out


# ---------------------------------------------------------------------------
# Custom fused DVE ops.
#   NOISY:   out = clip(s1 * (in0-s0) * ((in0-s0)^2 - in1), -1, 1)
#   SIGNADD: out = clip((in0 + in1) * s1, -1, 1)
# ---------------------------------------------------------------------------

_NOISY_OP_NAME = "NOISY_SIGN_PM1_ANT"
_SIGNADD_OP_NAME = "SIGN_ADD_PM1_ANT"


def _noisy_ref(in0, in1, c0, c1, c2):
    t = np.asarray(in0, np.float32) - np.float32(c0)
    r2 = np.asarray(in1, np.float32).reshape(t.shape)
    w = (t * (t * t - r2)) * np.float32(c1)
    return np.maximum(np.minimum(w, np.float32(1.0)), np.float32(-1.0))


def _signadd_ref(in0, in1, c0, c1, c2):
    v = (np.asarray(in0, np.float32)
         + np.asarray(in1, np.float32).reshape(np.shape(in0)))
    w = v * np.float32(c1)
    return np.maximum(np.minimum(w, np.float32(1.0)), np.float32(-1.0))


def _register_op(name, body_fn, ref):
    from concourse import dve_ops
    from concourse.dve_spec import (C0, C1, One, Spec, Src0, Src1, Zero,
                                    lower, maxx, minn)
    from concourse.dve_uop import DveOpSpec

    for op in dve_ops.OPS:
        if op.name == name:
            return op

    body = body_fn(C0, C1, One, Zero, Src0, Src1, maxx, minn)
    spec = Spec(body=body, reference=ref)

    row = dve_ops._CUSTOM_DVE_ROW_BASE + len(dve_ops.OPS)
    assert row < 0x20, "custom-DVE opcode rows exhausted"
    shas = {}
    for ver in ("v3", "v4"):
        d = DveOpSpec(name=name, opcode=row, uops=lower(spec, ver=ver),
                      rd1_en=True)
        shas[ver] = d.sha(ver)
    op = dve_ops.DveOp(name, spec, subdim=False, uops_sha=shas)
    dve_ops.OPS.append(op)
    dve_ops.CUSTOM_DVE_SPECS[name] = spec
    dve_ops._SUB_OPCODE_FOR_NAME[name] = row
    return op


def _noisy_body(C0, C1, One, Zero, Src0, Src1, maxx, minn):
    t = Src0 - C0
    w = (t * ((t * t) - Src1)) * C1
    return maxx(minn(w, One), Zero - One)


def _signadd_body(C0, C1, One, Zero, Src0, Src1, maxx, minn):
    w = (Src0 + Src1) * C1
    return maxx(minn(w, One), Zero - One)


NOISY_OP = _register_op(_NOISY_OP_NAME, _noisy_body, _noisy_ref)
SIGNADD_OP = _register_op(_SIGNADD_OP_NAME, _signadd_body, _signadd_ref)


def build_nc(repeat: int = 1):
    """Build the per-core Bass program (same program on all 8 cores)."""
    nc = bacc.Bacc("TRN2", target_bir_lowering=False, debug=False,
                   num_devices=N_CORES)

    xt = [nc.dram_tensor(f"xt{n}", [128, KXT, SL[n]], FP8,
                         kind="ExternalInput").ap() for n in range(NT)]
    w1 = [nc.dram_tensor(f"w1_{o}", [128, KA, 128], FP8,
                         kind="ExternalInput").ap() for o in range(OC)]
    a2 = [nc.dram_tensor(f"a2_{n}", [128, OC, SL[n]], FP8,
                         kind="ExternalInput").ap() for n in range(NT)]
    a3 = [nc.dram_tensor(f"a3_{n}", [128, OC, SL[n]], FP8,
                         kind="ExternalInput").ap() for n in range(NT)]
    w2 = nc.dram_tensor("w2", [128, KH, D_H], FP8, kind="ExternalInput").ap()
    w3 = nc.dram_tensor("w3", [128, KH, D_H], FP8, kind="ExternalInput").ap()
    w4 = nc.dram_tensor("w4", [128, KH, D_PAD4], FP8,
                        kind="ExternalInput").ap()
    c1 = nc.dram_tensor("c1", [128, OC], F32, kind="ExternalInput").ap()
    out = nc.dram_tensor("out", [D_OUT, BC], F32, kind="ExternalOutput").ap()

    with tile.TileContext(nc) as tc:
        with ExitStack() as ctx:
            consts = ctx.enter_context(tc.tile_pool(name="consts", bufs=1))
            panels = ctx.enter_context(tc.tile_pool(name="panels", bufs=1))
            bsb0 = ctx.enter_context(tc.tile_pool(name="bsb0", bufs=1))
            xtp = ctx.enter_context(tc.tile_pool(name="xtp", bufs=2))
            apool = ctx.enter_context(tc.tile_pool(name="apool", bufs=4))
            opool = ctx.enter_context(tc.tile_pool(name="opool", bufs=2))

            c1_t = consts.tile([128, OC], F32, tag="c1")
            w1_t = consts.tile([128, OC * KA, 128], FP8, tag="w1")
            w2_t = consts.tile([128, KH, D_H], FP8, tag="w2")
            w3_t = consts.tile([128, KH, D_H], FP8, tag="w3")
            w4_t = consts.tile([128, KH, D_PAD4], FP8, tag="w4")

            # +-1 activation panels, feature-major fp8.
            b1_t = panels.tile([128, KH, BC], FP8, tag="b1")
            b2_t = panels.tile([128, KH, BC], FP8, tag="b2")
            b3_t = panels.tile([128, KH, BC], FP8, tag="b3")

            # Priority order on the shared DMA engine: enough w1/xt to start
            # and continuously feed L1's A-phase, with the B-stream (xt
            # chunks KA..) and later consts behind it.
            # Front DMA, in deadline order on the shared DMA engine: c1
            # (stage(0) needs it early), then w1 blocks and xt0 A-chunk
            # pairs interleaved to track the PE's consumption rate, then the
            # B-stream and the remaining w1 blocks.
            xt_t: dict[int, object] = {}
            xt_t[0] = xtp.tile([128, KXT, SL[0]], FP8, tag="xt",
                               name="xt_t0")
            nc.sync.dma_start(c1_t[:], c1[:, :])
            nc.sync.dma_start(w1_t[:, 0:4, :], w1[0][:, 0:4, :])
            nc.sync.dma_start(xt_t[0][:, 0:2, :], xt[0][:, 0:2, :])
            nc.sync.dma_start(xt_t[0][:, 2:4, :], xt[0][:, 2:4, :])
            nc.sync.dma_start(w1_t[:, 4:12, :], w1[0][:, 4:12, :])
            nc.sync.dma_start(xt_t[0][:, 4:6, :], xt[0][:, 4:6, :])
            nc.sync.dma_start(xt_t[0][:, 6:8, :], xt[0][:, 6:8, :])
            nc.sync.dma_start(w1_t[:, 12:KA, :], w1[0][:, 12:, :])
            nc.sync.dma_start(xt_t[0][:, 8:10, :], xt[0][:, 8:10, :])
            nc.sync.dma_start(xt_t[0][:, 10:12, :], xt[0][:, 10:12, :])
            nc.sync.dma_start(w1_t[:, KA:KA + 10, :], w1[1][:, :10, :])
            nc.sync.dma_start(xt_t[0][:, 12:14, :], xt[0][:, 12:14, :])
            nc.sync.dma_start(w1_t[:, KA + 10:2 * KA, :], w1[1][:, 10:, :])
            nc.sync.dma_start(xt_t[0][:, 14:16, :], xt[0][:, 14:16, :])
            nc.sync.dma_start(w1_t[:, 2 * KA:2 * KA + 10, :],
                              w1[2][:, :10, :])
            nc.sync.dma_start(xt_t[0][:, 16:18, :], xt[0][:, 16:18, :])
            nc.sync.dma_start(w1_t[:, 2 * KA + 10:3 * KA, :],
                              w1[2][:, 10:, :])
            nc.sync.dma_start(xt_t[0][:, 18:KA, :], xt[0][:, 18:KA, :])
            nc.sync.dma_start(w1_t[:, 3 * KA:4 * KA, :], w1[3])
            nc.sync.dma_start(w1_t[:, 4 * KA:5 * KA, :], w1[4])
            nc.sync.dma_start(w1_t[:, 5 * KA:6 * KA, :], w1[5])
            nc.sync.dma_start(w1_t[:, 6 * KA:7 * KA, :], w1[6])
            nc.sync.dma_start(w1_t[:, 7 * KA:8 * KA, :], w1[7])
            nc.sync.dma_start(xt_t[0][:, KA:KA + 6, :], xt[0][:, KA:KA + 6, :])
            nc.sync.dma_start(xt_t[0][:, KA + 6:KA + 10, :],
                              xt[0][:, KA + 6:KA + 10, :])
            nc.sync.dma_start(xt_t[0][:, KA + 10:, :], xt[0][:, KA + 10:, :])
            nc.sync.dma_start(w4_t[:], w4[:, :, :])

            for _rep in range(repeat):
                with ExitStack() as rep_ctx:
                    l1a = rep_ctx.enter_context(
                        tc.tile_pool(name="l1a", bufs=2, space="PSUM"))
                    l1b = rep_ctx.enter_context(
                        tc.tile_pool(name="l1b", bufs=2, space="PSUM"))
                    l23 = rep_ctx.enter_context(
                        tc.tile_pool(name="l23", bufs=3, space="PSUM"))
                    l4ps = rep_ctx.enter_context(
                        tc.tile_pool(name="l4ps", bufs=1, space="PSUM"))

                    a2_t: dict[int, object] = {}
                    a3_t: dict[int, object] = {}
                    l4_pending = None  # (psum tile, slice index) across iters

                    def l4_finish(l4_fin):
                        """Last L4 DR pair + PSUM copy + output store."""
                        ps4, n4 = l4_fin
                        w4s = SL[n4]
                        s4 = slice(OFFS[n4], OFFS[n4] + w4s)
                        nc.tensor.matmul(ps4[:, :], w4_t[:, KH - 2:KH, :],
                                         b3_t[:, KH - 2:KH, s4],
                                         start=False, stop=True, perf_mode=DR)
                        ot = opool.tile([D_OUT, w4s], F32, tag="ot")
                        nc.scalar.activation(ot[:, :], ps4[:D_OUT, :],
                                             ACTF.Copy)
                        nc.sync.dma_start(out[:, s4], ot[:, :])

                    # Software pipeline, skewed one slice per layer.  Each
                    # iteration i runs 8 uniform rounds (one per out-chunk):
                    #   A(i,o) -> stage -> B(i,o-1) -> Sign | L2(i-1,o) +
                    #   noisy2 | L3(i-2,o) + noisy3 | L4(i-2) pair at odd o
                    # The stage writes 2^15*A + c1 INTO the group-B PSUM
                    # bank; the B pair-matmuls then accumulate on top
                    # (start=False), so b1 = Sign(PSUM) is a plain ACT op
                    # and the DVE runs only the two noisy-sign ops per
                    # round (~1.3us DVE vs ~2.7us PE: always PE-bound).
                    # B trails A by one round so the stage (ACT) hides
                    # under A(o+1).  L4(i-2)'s last pair + PSUM copy +
                    # store run in the next iteration's round 4.
                    for i in range(NT + 3):
                        l4_fin, l4_pending = l4_pending, None

                        # --- DMA prefetch for this iteration ---
                        if i + 1 < NT:
                            xt_t[i + 1] = xtp.tile([128, KXT, SL[i + 1]],
                                                   FP8, tag="xt",
                                                   name=f"xt_t{i + 1}")
                            # in pieces: the next iteration's first rounds
                            # can start on the A-stream prefix while the
                            # rest is still in flight
                            nc.sync.dma_start(xt_t[i + 1][:, :10, :],
                                              xt[i + 1][:, :10, :])
                            nc.sync.dma_start(xt_t[i + 1][:, 10:KA, :],
                                              xt[i + 1][:, 10:KA, :])
                            nc.sync.dma_start(xt_t[i + 1][:, KA:, :],
                                              xt[i + 1][:, KA:, :])
                        if i == 0:
                            nc.sync.dma_start(w2_t[:], w2[:, :, :])
                        if i == 1:
                            nc.sync.dma_start(w3_t[:], w3[:, :, :])
                        if i < NT:
                            t_a2 = apool.tile([128, OC, SL[i]], FP8,
                                              tag="a2")
                            nc.sync.dma_start(t_a2[:], a2[i])
                            a2_t[i] = t_a2
                        if 1 <= i <= NT:
                            t_a3 = apool.tile([128, OC, SL[i - 1]], FP8,
                                              tag="a3")
                            nc.sync.dma_start(t_a3[:], a3[i - 1])
                            a3_t[i - 1] = t_a3

                        n1, n2, n3 = i, i - 1, i - 2
                        if n1 < NT:
                            ws1 = SL[n1]
                            s1 = slice(OFFS[n1], OFFS[n1] + ws1)
                        if 0 <= n2 < NT:
                            ws2 = SL[n2]
                            s2 = slice(OFFS[n2], OFFS[n2] + ws2)
                        if 0 <= n3 < NT:
                            ws3 = SL[n3]
                            s3 = slice(OFFS[n3], OFFS[n3] + ws3)

                        # Iteration 0 is DMA-bound: the front-loaded w1/xt0
                        # stream is consumed 1:1 as it lands, and the merged
                        # rounds below would need the B-stream ~7us earlier
                        # than the DMA can deliver it.  So slice 0 runs the
                        # two-phase order instead: all A-chains (stage to an
                        # SBUF tile), then all B-chains with a fused DVE
                        # sign-add, giving the B-stream DMA until the
                        # A-phase's end to arrive.
                        if i == 0:
                            bsa = bsb0.tile([128, OC, ws1], F32, tag="bsb")
                            for o in range(OC):
                                pa = l1a.tile([128, ws1], F32, tag="mma")
                                for kp in range(KA // 2):
                                    nc.tensor.matmul(
                                        pa[:, :],
                                        w1_t[:, o * KA + 2 * kp:
                                             o * KA + 2 * kp + 2, :],
                                        xt_t[0][:, 2 * kp:2 * kp + 2, :],
                                        start=(kp == 0),
                                        stop=(kp == KA // 2 - 1),
                                        perf_mode=DR,
                                    )
                                nc.scalar.activation(
                                    bsa[:, o, :], pa[:, :], ACTF.Identity,
                                    scale=ASCALE, bias=c1_t[:, o:o + 1])
                            for o in range(OC):
                                pb = l1b.tile([128, ws1], F32, tag="mmb")
                                for kp in range(KB // 2):
                                    nc.tensor.matmul(
                                        pb[:, :],
                                        w1_t[:, o * KA + 2 * kp:
                                             o * KA + 2 * kp + 2, :],
                                        xt_t[0][:, KA + 2 * kp:
                                                 KA + 2 * kp + 2, :],
                                        start=(kp == 0),
                                        stop=(kp == KB // 2 - 1),
                                        perf_mode=DR,
                                    )
                                # b1 = clip((B + bsb)*2^35) = sign(h1-mu1)
                                nc.vector._custom_dve(
                                    SIGNADD_OP, out=b1_t[:, o, s1],
                                    in0=pb[:, :], in1=bsa[:, o, :],
                                    s0=0.0, s1=float(2.0 ** 35))
                            continue

                        def l2_round(o2, n2, s2, ws2):
                            p2 = l23.tile([128, ws2], F32, tag="mm23",
                                          name="p2")
                            for kp in range(KH // 2):
                                nc.tensor.matmul(
                                    p2[:, :],
                                    w2_t[:, 2 * kp:2 * kp + 2,
                                         o2 * 128:(o2 + 1) * 128],
                                    b1_t[:, 2 * kp:2 * kp + 2, s2],
                                    start=(kp == 0),
                                    stop=(kp == KH // 2 - 1),
                                    perf_mode=DR,
                                )
                            nc.vector._custom_dve(
                                NOISY_OP, out=b2_t[:, o2, s2],
                                in0=p2[:, :], in1=a2_t[n2][:, o2, :],
                                s0=0.25, s1=32.0)

                        def l1_b_chain(o, pb):
                            """Group-B pairs accumulate onto the staged
                            PSUM bank (start=False), then b1 = Sign(PSUM)."""
                            for kp in range(KB // 2):
                                # start=False on an ACT-initialized bank is
                                # intentional: accumulate onto 2^15*A + c1.
                                nc.tensor.matmul(
                                    pb[:, :],
                                    w1_t[:, o * KA + 2 * kp:
                                         o * KA + 2 * kp + 2, :],
                                    xt_t[n1][:, KA + 2 * kp:
                                             KA + 2 * kp + 2, :],
                                    start=False,
                                    stop=(kp == KB // 2 - 1),
                                    perf_mode=DR,
                                    skip_group_check=True,
                                )
                            nc.scalar.activation(
                                b1_t[:, o, s1], pb[:, :], ACTF.Sign)

                        pb_prev = None  # (o, staged PSUM tile) from last round
                        for o in range(OC):
                            if n1 < NT:
                                pa = l1a.tile([128, ws1], F32, tag="mma")
                                for kp in range(KA // 2):
                                    nc.tensor.matmul(
                                        pa[:, :],
                                        w1_t[:, o * KA + 2 * kp:
                                             o * KA + 2 * kp + 2, :],
                                        xt_t[n1][:, 2 * kp:2 * kp + 2, :],
                                        start=(kp == 0),
                                        stop=(kp == KA // 2 - 1),
                                        perf_mode=DR,
                                    )
                                # PSUM init for group B: 2^15*A + c1
                                pb = l1b.tile([128, ws1], F32, tag="mmb")
                                nc.scalar.activation(
                                    pb[:, :], pa[:, :], ACTF.Identity,
                                    scale=ASCALE, bias=c1_t[:, o:o + 1])
                                if pb_prev is not None:
                                    l1_b_chain(*pb_prev)
                                pb_prev = (o, pb)
                            if o == 3 and l4_fin is not None:
                                l4_finish(l4_fin)
                                l4_fin = None
                            # Iteration 1's L2 rounds lag by 2 so the w2/a2
                            # DMAs (queued behind the front-load) land first.
                            o2 = o - 2 if i == 1 else o
                            if 0 <= n2 < NT - 1 and o2 >= 0:
                                l2_round(o2, n2, s2, ws2)
                            if 0 <= n3 < NT:
                                p3 = l23.tile([128, ws3], F32, tag="mm23",
                                              name="p3")
                                for kp in range(KH // 2):
                                    nc.tensor.matmul(
                                        p3[:, :],
                                        w3_t[:, 2 * kp:2 * kp + 2,
                                             o * 128:(o + 1) * 128],
                                        b2_t[:, 2 * kp:2 * kp + 2, s3],
                                        start=(kp == 0),
                                        stop=(kp == KH // 2 - 1),
                                        perf_mode=DR,
                                    )
                                nc.vector._custom_dve(
                                    NOISY_OP, out=b3_t[:, o, s3],
                                    in0=p3[:, :], in1=a3_t[n3][:, o, :],
                                    s0=0.25, s1=32.0)
                                if o >= 3 and o % 2 == 1:
                                    kp = (o - 3) // 2
                                    if kp == 0:
                                        ps4 = l4ps.tile([D_PAD4, ws3], F32,
                                                        tag="mm4")
                                        l4_pending = (ps4, n3)
                                    nc.tensor.matmul(
                                        ps4[:, :],
                                        w4_t[:, 2 * kp:2 * kp + 2, :],
                                        b3_t[:, 2 * kp:2 * kp + 2, s3],
                                        start=(kp == 0), stop=False,
                                        perf_mode=DR)
                        if pb_prev is not None:
                            l1_b_chain(*pb_prev)
                        if i == 1 and 0 <= n2 < NT - 1:
                            for o2 in (OC - 2, OC - 1):  # lagged catch-up
                                l2_round(o2, n2, s2, ws2)
                        if l4_fin is not None:  # iterations with empty loop
                            l4_finish(l4_fin)

                        # --- gamma-phase (last L1 iteration only): run the
                        # final slice's L2 immediately, so its noisy-sign DVE
                        # work overlaps the remaining tail PE work instead of
                        # serializing after it.
                        if i == NT - 1:
                            sg = s1
                            for o in range(OC):
                                p2 = l23.tile([128, ws1], F32, tag="mm23",
                                              name="p2g")
                                for kp in range(KH // 2):
                                    nc.tensor.matmul(
                                        p2[:, :],
                                        w2_t[:, 2 * kp:2 * kp + 2,
                                             o * 128:(o + 1) * 128],
                                        b1_t[:, 2 * kp:2 * kp + 2, sg],
                                        start=(kp == 0),
                                        stop=(kp == KH // 2 - 1),
                                        perf_mode=DR,
                                    )
                                nc.vector._custom_dve(
                                    NOISY_OP, out=b2_t[:, o, sg],
                                    in0=p2[:, :], in1=a2_t[n1][:, o, :],
                                    s0=0.25, s1=32.0)

    nc.compile()
    return nc


_NC_CACHE: dict[int, object] = {}


def _get_nc(repeat: int = 1):
    if repeat not in _NC_CACHE:
        _NC_CACHE[repeat] = build_nc(repeat)
    return _NC_CACHE[repeat]


def make_in_maps(x, u2, u3, W1, W2, W3, W4, **_unused):
    """Host preprocessing -> per-core input dicts."""
    fp8_np = mybir.dt.np(FP8)

    x = np.ascontiguousarray(np.asarray(x, dtype=np.float32))
    W1b = np.sign(np.asarray(W1, dtype=np.float32))
    # mean(h1, axis=0) = sign(W1) @ mean(x, axis=0), in float64; negated and
    # pre-scaled so the device computes sign(2^21 h1 + bias).
    mu1 = (W1b.astype(np.float64) @ x.mean(axis=0, dtype=np.float64)).astype(
        np.float32)
    c1 = np.ascontiguousarray(
        (np.float32(-XSCALE) * mu1).reshape(OC, 128).T)  # [128, OC]

    # balanced base-32 fixed-point split: round(x*2^21) = sum d_j 32^(4-j),
    # digits d_j in [-16, 16] - every one exact in fp8e4m3.
    assert np.abs(x).max() * XSCALE < 16.5 * (32 ** 5 - 1) / 31, "x overflow"
    n = np.rint(x.T.astype(np.float64) * XSCALE).astype(np.int64)  # [784, B]
    digs = []
    for _ in range(5):
        d = ((n + 16) % 32) - 16
        n = (n - d) >> 5
        digs.append(d)                      # LSB first: d4, d3, d2, d1, d0
    assert np.all(n == 0), "digit overflow"
    d4, d3, d2, d1, d0 = digs

    # xt slice layout [128, KXT, SW]: A stream (d0,d1,d2 = 2352 rows, pad
    # to KA*128) then B stream (d3,d4 = 1568 rows, pad to KB*128).
    xt_all = np.zeros((KXT * 128, B), dtype=fp8_np)
    for j, d in enumerate((d0, d1, d2)):
        xt_all[j * D_IN:(j + 1) * D_IN] = d.astype(np.float32).astype(fp8_np)
    for j, d in enumerate((d3, d4)):
        r = KA * 128 + j * D_IN
        xt_all[r:r + D_IN] = d.astype(np.float32).astype(fp8_np)

    # stored weight stream per out-chunk: [32*w, w, w/32] (2352 rows + pad),
    # shared by groups A and B (B reads the 2/3 prefix).
    w1p = np.zeros((KA * 128, D_H), dtype=fp8_np)
    for j, sc in enumerate((32.0, 1.0, 1.0 / 32.0)):
        w1p[j * D_IN:(j + 1) * D_IN] = (
            W1b.T * np.float32(sc)).astype(fp8_np)
    # [o][p][k][m]: one contiguous DMA per 128-feature output block.
    w1_blocks = np.ascontiguousarray(
        w1p.reshape(KA, 128, OC, 128).transpose(2, 1, 0, 3))

    pt = _prob_table()
    t2tab = _t2_table(fp8_np)
    a2f = t2tab[(_flip_thresholds(np.asarray(u2), pt) // 2)]   # [B, 1024]
    a3f = t2tab[(_flip_thresholds(np.asarray(u3), pt) // 2)]

    def _hidden_w(w, sc):
        wt = (np.sign(np.asarray(w, np.float32)) * np.float32(sc)
              ).T.astype(fp8_np)                               # [K, M]
        return np.ascontiguousarray(
            wt.reshape(KH, 128, wt.shape[1]).transpose(1, 0, 2))

    w2t = _hidden_w(W2, 0.5)               # [128, 8, 1024], +-0.5
    w3t = _hidden_w(W3, 0.5)
    w4t = _hidden_w(W4, 1.0)               # [128, 8, 10]
    w4p = np.zeros((128, KH, D_PAD4), dtype=fp8_np)
    w4p[:, :, :D_OUT] = w4t
    w4t = w4p

    in_maps = []
    for c in range(N_CORES):
        sl = slice(c * BC, (c + 1) * BC)
        m = {"w2": w2t, "w3": w3t, "w4": w4t, "c1": c1}
        xc = xt_all[:, sl].reshape(KXT, 128, BC)  # [k, p, col]
        for nn in range(NT):
            cs = slice(OFFS[nn], OFFS[nn] + SL[nn])
            m[f"xt{nn}"] = np.ascontiguousarray(
                xc[:, :, cs].transpose(1, 0, 2))
        for o in range(OC):
            m[f"w1_{o}"] = w1_blocks[o]
        for nm, tab in (("a2", a2f), ("a3", a3f)):
            rc = tab.T[:, sl].astype(fp8_np).reshape(OC, 128, BC)
            for nn in range(NT):
                cs = slice(OFFS[nn], OFFS[nn] + SL[nn])
                m[f"{nm}_{nn}"] = np.ascontiguousarray(
                    rc[:, :, cs].transpose(1, 0, 2))
        in_maps.append(m)
    return in_maps


def kernel(x, u2, u3, W1, W2, W3, W4,
           g1=None, b1=None, g2=None, b2=None, g3=None, b3=None):
    for g in (g1, g2, g3):
        assert g is None or np.all(np.asarray(g) > 0), "kernel assumes g > 0"
    for b in (b1, b2, b3):
        assert b is None or np.all(np.asarray(b) == 0), "kernel assumes b == 0"

    nc = _get_nc(repeat=1)
    in_maps = make_in_maps(x, u2, u3, W1, W2, W3, W4)
    res = run_bass_kernel_spmd(nc, in_maps, core_ids=list(range(N_CORES)))

    out = np.empty((B, D_OUT), dtype=np.float32)
    for c in range(N_CORES):
        out[c * BC:(c + 1) * BC, :] = res.results[c]["out"].T
    return out


# revision 37
# speedup vs baseline: 1.1060x; 1.0451x over previous
"""Bass/Trainium2 kernel for a binarized NN (BNN) forward pass, data-parallel
over 8 NeuronCores.

Reference semantics (fp32):
    h1 = x @ sign(W1).T;  b1 = sign(h1 - mean(h1, axis=0))        # g=1, b=0
    h2 = b1 @ sign(W2).T; b2 = noisy_sign(h2, u2)                  # BN+sign is
    h3 = b2 @ sign(W3).T; b3 = noisy_sign(h3, u3)                  # identity on +-1
    out = b3 @ sign(W4).T

Implementation summary (all arithmetic exact small-integer, as in the
fixed-point analysis below):
  * Layer 1 contracts a balanced base-32 fixed-point split of x:
    round(x*2^21) = sum_j d_j 32^(4-j) with digits d_j in [-16,16], every
    one exact in fp8e4m3.  Two PSUM groups share one stored weight stream
    [32*w, w, w/32] (w = sign(W1)): group A moves digits (d0,d1,d2) over
    the full stream, group B moves (d3,d4) over its 2/3 prefix, so
    h1*2^21 = 2^15*A + B.  All products and DoubleRow pair-sums are
    integers scaled by powers of two within the pair-adder's ~12-bit
    mantissa, and each group's PSUM total stays under 2^24/granularity,
    so accumulation is exact.  17 DR pair-matmuls per output chunk
    (vs 20 for the 6-nibble scheme).
  * ACT stages bsb = 2^15*A + c1 (c1 = -2^21*mean(h1), computed on host in
    float64); a single custom DVE op then emits
    b1 = clip((B + bsb) * 2^35, -1, 1) = sign(h1 - mu1) directly.  The
    only inexactness is one fp32 rounding in the stage/add (~2^-19 of h1)
    plus the 2^-22 input quantization - both far below the reference's
    own matmul rounding scale.
  * W2/W3 ship as +-0.5 so PSUM holds h' = h/2 (an exact integer <= 512).
    The stochastic flip (u < 0.5 exp(-h^2/50), |h| <= 50) depends only on
    A(u) = smallest even a with p(a) <= u: flip <=> |h| < A.  With
    t = h' - 1/4 and T2 ~ ((A-1)/2)^2 (fp8-rounded inside its decision
    margin, -1 when A = 0),
        noisy = clip(32 * t * (t*t - T2), -1, 1)
    equals the exact noisy sign for every integer h, so the u-derived
    tables ship as fp8, halving their HBM traffic vs bf16.
  * b in {+-1} and sign(W4) in {+-1} make the output exact integers.

Layout is feature-major: activations live as [features(partitions),
batch(free)].  Batch 16384 is sharded 2048/core; each core pipelines four
512-column slices through all four layers, skewed one slice per layer.
Each iteration has an A-phase (8 L1 group-A chains + ACT stages, no DVE
dependencies - DVE backlog from the previous iteration drains under it)
and a B-phase (L1 group-B chains + SignAdd, interleaved with the previous
slices' L2/L3/L4 chains and noisy-sign ops).
"""

from contextlib import ExitStack

import numpy as np

import concourse.bass as bass  # noqa: F401
import concourse.tile as tile
from concourse import bacc, mybir
from concourse.bass_utils import run_bass_kernel_spmd

F32 = mybir.dt.float32
BF16 = mybir.dt.bfloat16
FP8 = mybir.dt.float8e4
ACTF = mybir.ActivationFunctionType
DR = mybir.MatmulPerfMode.DoubleRow

N_CORES = 8
B = 16384                 # full batch
BC = B // N_CORES         # batch per core
D_IN = 784                # layer-1 input features
D_H = 1024                # hidden features
D_OUT = 10                # output features
D_PAD4 = 16               # L4 stationary dim padded for DoubleRow
XSCALE = float(2 ** 21)   # fixed-point scale of |x|
ASCALE = float(2 ** 15)   # h1*2^21 = 2^15*A + B
KH = D_H // 128           # 8 k-chunks for hidden layers
OC = D_H // 128           # 8 output-feature chunks
# Layer-1 chunk geometry: A stream = digits (d0,d1,d2) = 2352 rows ->
# 10 DR pair-calls (20 chunks padded); B stream = (d3,d4) = 1568 rows ->
# 7 pair-calls (14 chunks padded).
KA = 20                   # A-stream chunks (incl pad), 10 pair-calls
KB = 14                   # B-stream chunks (incl pad), 7 pair-calls
KXT = KA + KB             # chunks per xt slice
# Batch-column slice widths: 512 amortizes per-op overheads and keeps DMA
# runs >= 512B; the last two slices narrow to 256 to shrink the DVE-bound
# pipeline drain (the tail's noisy-sign ops can't hide under L1 PE work).
SL = [512, 512, 512, 256, 256]
OFFS = [sum(SL[:i]) for i in range(len(SL))]
NT = len(SL)
assert sum(SL) == BC

# float32(0.5*exp(-(a*a)/50)) for a = 0,2,...,50 (bit-exact fallback table).
_PTABLE_BITS = [
    0x3F000000, 0x3EEC515A, 0x3EB9E4E3, 0x3E79375C, 0x3E0E5ACB, 0x3D8A9501,
    0x3CE5ED93, 0x3C2289CB, 0x3B43D285, 0x3A4909DD, 0x392FE09E, 0x38031DFC,
    0x36A696B8, 0x35345CD8, 0x33A6674D, 0x3202D2C5, 0x302F4A31, 0x2E4824C7,
    0x2C42BB52, 0x2A2173E9, 0x27E4229E, 0x258959AD, 0x230CEE5E, 0x207672F6,
    0x1DB79FE2, 0x1AE92B5E,
]


def _prob_table() -> np.ndarray:
    """p(a) for a = 0,2,...,50, bit-matching the reference's jnp.exp."""
    try:
        import jax.numpy as jnp

        a = np.arange(0, 51, 2, dtype=np.float32)
        p = np.asarray(0.5 * jnp.exp(-(jnp.asarray(a) * a) / (2.0 * 5.0**2)),
                       dtype=np.float32)
        if p.shape == (26,) and np.all(np.diff(p) < 0):
            return p
    except Exception:
        pass
    return np.array(_PTABLE_BITS, dtype=np.uint32).view(np.float32)


def _flip_thresholds(u: np.ndarray, ptable: np.ndarray) -> np.ndarray:
    """A(u): flip <=> |h| < A. A = 52 - 2 * #{a : p(a) <= u}."""
    tab = ptable[::-1].copy()  # ascending: p(50), p(48), ..., p(0)
    idx = np.searchsorted(tab, u, side="right")
    return (52 - 2 * idx).astype(np.int64)


def _t2_entry(A: int, fp8_np) -> float:
    """fp8 threshold T2 for even A: separates t^2 = (h'-1/4)^2 at integer
    h' into flip (|h'| < A/2) vs keep.  Must lie strictly inside
    ((A/2-0.75)^2, (A/2-0.25)^2) after fp8 rounding, with s1=32 margins."""
    if A == 0:
        return -1.0
    lo, hi = (A / 2 - 0.75) ** 2, (A / 2 - 0.25) ** 2
    cand = float(np.asarray((A / 2 - 0.5) ** 2, np.float32).astype(fp8_np))
    if not (lo < cand < hi):
        # scan fp8-representable values inside the interval
        for v in np.linspace(lo, hi, 64)[1:-1]:
            c = float(np.asarray(v, np.float32).astype(fp8_np))
            if lo < c < hi:
                cand = c
                break
        else:
            raise AssertionError(f"no fp8 threshold for A={A}")
    return cand


def _t2_table(fp8_np) -> np.ndarray:
    """T2 per A-index (A = 0,2,...,52), fp8 values as fp32, capped at 448."""
    out = np.empty(27, np.float32)
    for i in range(27):
        A = 2 * i
        if A >= 34:
            # fp8e4m3 (IEEE, max 240) can't hold larger thresholds.
            # P(A >= 34) = 6.3e-10/element: ~0.02 occurrences expected in
            # the whole problem, and an error additionally needs |h|>=32.
            out[i] = 240.0
        else:
            out[i] = _t2_entry(A, fp8_np)
    # margin check: |32*t*(t^2-T2)| >= 1 with correct sign, h' integer
    hp = np.arange(-600, 601)
    t = hp - 0.25
    for i in range(27):
        A = 2 * i
        w = 32.0 * t * (t * t - out[i])
        assert np.all(np.abs(w) >= 1.0), (A, np.abs(w).min())
        if A >= 34:
            continue
        flip = np.abs(2 * hp) < A
        s = np.where(hp > 0, 1.0, -1.0)
        want = np.where(flip, -s, s)
        assert np.all(np.clip(w, -1.0, 1.0) == want), (A,)
    return out


# ---------------------------------------------------------------------------
# Custom fused DVE ops.
#   NOISY:   out = clip(s1 * (in0-s0) * ((in0-s0)^2 - in1), -1, 1)
#   SIGNADD: out = clip((in0 + in1) * s1, -1, 1)
# ---------------------------------------------------------------------------

_NOISY_OP_NAME = "NOISY_SIGN_PM1_ANT"
_SIGNADD_OP_NAME = "SIGN_ADD_PM1_ANT"


def _noisy_ref(in0, in1, c0, c1, c2):
    t = np.asarray(in0, np.float32) - np.float32(c0)
    r2 = np.asarray(in1, np.float32).reshape(t.shape)
    w = (t * (t * t - r2)) * np.float32(c1)
    return np.maximum(np.minimum(w, np.float32(1.0)), np.float32(-1.0))


def _signadd_ref(in0, in1, c0, c1, c2):
    v = (np.asarray(in0, np.float32)
         + np.asarray(in1, np.float32).reshape(np.shape(in0)))
    w = v * np.float32(c1)
    return np.maximum(np.minimum(w, np.float32(1.0)), np.float32(-1.0))


def _register_op(name, body_fn, ref):
    from concourse import dve_ops
    from concourse.dve_spec import (C0, C1, One, Spec, Src0, Src1, Zero,
                                    lower, maxx, minn)
    from concourse.dve_uop import DveOpSpec

    for op in dve_ops.OPS:
        if op.name == name:
            return op

    body = body_fn(C0, C1, One, Zero, Src0, Src1, maxx, minn)
    spec = Spec(body=body, reference=ref)

    row = dve_ops._CUSTOM_DVE_ROW_BASE + len(dve_ops.OPS)
    assert row < 0x20, "custom-DVE opcode rows exhausted"
    shas = {}
    for ver in ("v3", "v4"):
        d = DveOpSpec(name=name, opcode=row, uops=lower(spec, ver=ver),
                      rd1_en=True)
        shas[ver] = d.sha(ver)
    op = dve_ops.DveOp(name, spec, subdim=False, uops_sha=shas)
    dve_ops.OPS.append(op)
    dve_ops.CUSTOM_DVE_SPECS[name] = spec
    dve_ops._SUB_OPCODE_FOR_NAME[name] = row
    return op


def _noisy_body(C0, C1, One, Zero, Src0, Src1, maxx, minn):
    t = Src0 - C0
    w = (t * ((t * t) - Src1)) * C1
    return maxx(minn(w, One), Zero - One)


def _signadd_body(C0, C1, One, Zero, Src0, Src1, maxx, minn):
    w = (Src0 + Src1) * C1
    return maxx(minn(w, One), Zero - One)


NOISY_OP = _register_op(_NOISY_OP_NAME, _noisy_body, _noisy_ref)
SIGNADD_OP = _register_op(_SIGNADD_OP_NAME, _signadd_body, _signadd_ref)


def build_nc(repeat: int = 1):
    """Build the per-core Bass program (same program on all 8 cores)."""
    nc = bacc.Bacc("TRN2", target_bir_lowering=False, debug=False,
                   num_devices=N_CORES)

    xt = [nc.dram_tensor(f"xt{n}", [128, KXT, SL[n]], FP8,
                         kind="ExternalInput").ap() for n in range(NT)]
    w1 = [nc.dram_tensor(f"w1_{o}", [128, KA, 128], FP8,
                         kind="ExternalInput").ap() for o in range(OC)]
    a2 = [nc.dram_tensor(f"a2_{n}", [128, OC, SL[n]], FP8,
                         kind="ExternalInput").ap() for n in range(NT)]
    a3 = [nc.dram_tensor(f"a3_{n}", [128, OC, SL[n]], FP8,
                         kind="ExternalInput").ap() for n in range(NT)]
    w2 = nc.dram_tensor("w2", [128, KH, D_H], FP8, kind="ExternalInput").ap()
    w3 = nc.dram_tensor("w3", [128, KH, D_H], FP8, kind="ExternalInput").ap()
    w4 = nc.dram_tensor("w4", [128, KH, D_PAD4], FP8,
                        kind="ExternalInput").ap()
    c1 = nc.dram_tensor("c1", [128, OC], F32, kind="ExternalInput").ap()
    out = nc.dram_tensor("out", [D_OUT, BC], F32, kind="ExternalOutput").ap()

    with tile.TileContext(nc) as tc:
        with ExitStack() as ctx:
            consts = ctx.enter_context(tc.tile_pool(name="consts", bufs=1))
            panels = ctx.enter_context(tc.tile_pool(name="panels", bufs=1))
            bsb0 = ctx.enter_context(tc.tile_pool(name="bsb0", bufs=1))
            xtp = ctx.enter_context(tc.tile_pool(name="xtp", bufs=2))
            apool = ctx.enter_context(tc.tile_pool(name="apool", bufs=4))
            opool = ctx.enter_context(tc.tile_pool(name="opool", bufs=2))

            c1_t = consts.tile([128, OC], F32, tag="c1")
            w1_t = consts.tile([128, OC * KA, 128], FP8, tag="w1")
            w2_t = consts.tile([128, KH, D_H], FP8, tag="w2")
            w3_t = consts.tile([128, KH, D_H], FP8, tag="w3")
            w4_t = consts.tile([128, KH, D_PAD4], FP8, tag="w4")

            # +-1 activation panels, feature-major fp8.
            b1_t = panels.tile([128, KH, BC], FP8, tag="b1")
            b2_t = panels.tile([128, KH, BC], FP8, tag="b2")
            b3_t = panels.tile([128, KH, BC], FP8, tag="b3")

            # Priority order on the shared DMA engine: enough w1/xt to start
            # and continuously feed L1's A-phase, with the B-stream (xt
            # chunks KA..) and later consts behind it.
            # Front DMA.  Every copy pays ~625ns of serialized HWDGE
            # descriptor generation before its transfer, so the count is
            # kept low: two tiny copies unblock the first matmul ~2.4us in,
            # then large deadline-ordered pieces keep the L1 A-chains fed.
            xt_t: dict[int, object] = {}
            xt_t[0] = xtp.tile([128, KXT, SL[0]], FP8, tag="xt",
                               name="xt_t0")
            nc.sync.dma_start(w1_t[:, 0:2, :], w1[0][:, 0:2, :])
            nc.sync.dma_start(xt_t[0][:, 0:2, :], xt[0][:, 0:2, :])
            nc.sync.dma_start(w1_t[:, 2:KA, :], w1[0][:, 2:, :])
            nc.sync.dma_start(xt_t[0][:, 2:8, :], xt[0][:, 2:8, :])
            nc.sync.dma_start(c1_t[:], c1[:, :])
            nc.sync.dma_start(xt_t[0][:, 8:14, :], xt[0][:, 8:14, :])
            nc.sync.dma_start(w1_t[:, KA:2 * KA, :], w1[1])
            nc.sync.dma_start(xt_t[0][:, 14:KA, :], xt[0][:, 14:KA, :])
            nc.sync.dma_start(w1_t[:, 2 * KA:3 * KA, :], w1[2])
            nc.sync.dma_start(w1_t[:, 3 * KA:4 * KA, :], w1[3])
            nc.sync.dma_start(w1_t[:, 4 * KA:5 * KA, :], w1[4])
            nc.sync.dma_start(w1_t[:, 5 * KA:6 * KA, :], w1[5])
            nc.sync.dma_start(w1_t[:, 6 * KA:7 * KA, :], w1[6])
            nc.sync.dma_start(w1_t[:, 7 * KA:8 * KA, :], w1[7])
            nc.sync.dma_start(xt_t[0][:, KA:KA + 7, :], xt[0][:, KA:KA + 7, :])
            nc.sync.dma_start(xt_t[0][:, KA + 7:, :], xt[0][:, KA + 7:, :])
            nc.sync.dma_start(w4_t[:], w4[:, :, :])

            for _rep in range(repeat):
                with ExitStack() as rep_ctx:
                    l1a = rep_ctx.enter_context(
                        tc.tile_pool(name="l1a", bufs=2, space="PSUM"))
                    l1b = rep_ctx.enter_context(
                        tc.tile_pool(name="l1b", bufs=2, space="PSUM"))
                    l23 = rep_ctx.enter_context(
                        tc.tile_pool(name="l23", bufs=3, space="PSUM"))
                    l4ps = rep_ctx.enter_context(
                        tc.tile_pool(name="l4ps", bufs=1, space="PSUM"))

                    a2_t: dict[int, object] = {}
                    a3_t: dict[int, object] = {}
                    l4_pending = None  # (psum tile, slice index) across iters

                    def l4_finish(l4_fin):
                        """Last L4 DR pair + PSUM copy + output store."""
                        ps4, n4 = l4_fin
                        w4s = SL[n4]
                        s4 = slice(OFFS[n4], OFFS[n4] + w4s)
                        nc.tensor.matmul(ps4[:, :], w4_t[:, KH - 2:KH, :],
                                         b3_t[:, KH - 2:KH, s4],
                                         start=False, stop=True, perf_mode=DR)
                        ot = opool.tile([D_OUT, w4s], F32, tag="ot")
                        nc.scalar.activation(ot[:, :], ps4[:D_OUT, :],
                                             ACTF.Copy)
                        nc.sync.dma_start(out[:, s4], ot[:, :])

                    # Software pipeline, skewed one slice per layer.  Each
                    # iteration i runs 8 uniform rounds (one per out-chunk):
                    #   A(i,o) -> stage -> B(i,o-1) -> Sign | L2(i-1,o) +
                    #   noisy2 | L3(i-2,o) + noisy3 | L4(i-2) pair at odd o
                    # The stage writes 2^15*A + c1 INTO the group-B PSUM
                    # bank; the B pair-matmuls then accumulate on top
                    # (start=False), so b1 = Sign(PSUM) is a plain ACT op
                    # and the DVE runs only the two noisy-sign ops per
                    # round (~1.3us DVE vs ~2.7us PE: always PE-bound).
                    # B trails A by one round so the stage (ACT) hides
                    # under A(o+1).  L4(i-2)'s last pair + PSUM copy +
                    # store run in the next iteration's round 4.
                    for i in range(NT + 3):
                        l4_fin, l4_pending = l4_pending, None

                        # --- DMA prefetch for this iteration ---
                        if i + 1 < NT:
                            xt_t[i + 1] = xtp.tile([128, KXT, SL[i + 1]],
                                                   FP8, tag="xt",
                                                   name=f"xt_t{i + 1}")
                            # in pieces: the next iteration's first rounds
                            # can start on the A-stream prefix while the
                            # rest is still in flight
                            nc.sync.dma_start(xt_t[i + 1][:, :10, :],
                                              xt[i + 1][:, :10, :])
                            nc.sync.dma_start(xt_t[i + 1][:, 10:KA, :],
                                              xt[i + 1][:, 10:KA, :])
                            nc.sync.dma_start(xt_t[i + 1][:, KA:, :],
                                              xt[i + 1][:, KA:, :])
                        if i == 0:
                            nc.sync.dma_start(w2_t[:], w2[:, :, :])
                        if i == 1:
                            nc.sync.dma_start(w3_t[:], w3[:, :, :])
                        if i < NT:
                            t_a2 = apool.tile([128, OC, SL[i]], FP8,
                                              tag="a2")
                            nc.sync.dma_start(t_a2[:], a2[i])
                            a2_t[i] = t_a2
                        if 1 <= i <= NT:
                            t_a3 = apool.tile([128, OC, SL[i - 1]], FP8,
                                              tag="a3")
                            nc.sync.dma_start(t_a3[:], a3[i - 1])
                            a3_t[i - 1] = t_a3

                        n1, n2, n3 = i, i - 1, i - 2
                        if n1 < NT:
                            ws1 = SL[n1]
                            s1 = slice(OFFS[n1], OFFS[n1] + ws1)
                        if 0 <= n2 < NT:
                            ws2 = SL[n2]
                            s2 = slice(OFFS[n2], OFFS[n2] + ws2)
                        if 0 <= n3 < NT:
                            ws3 = SL[n3]
                            s3 = slice(OFFS[n3], OFFS[n3] + ws3)

                        # Iteration 0 is DMA-bound: the front-loaded w1/xt0
                        # stream is consumed 1:1 as it lands, and the merged
                        # rounds below would need the B-stream ~7us earlier
                        # than the DMA can deliver it.  So slice 0 runs the
                        # two-phase order instead: all A-chains (stage to an
                        # SBUF tile), then all B-chains with a fused DVE
                        # sign-add, giving the B-stream DMA until the
                        # A-phase's end to arrive.
                        if i == 0:
                            bsa = bsb0.tile([128, OC, ws1], F32, tag="bsb")
                            for o in range(OC):
                                pa = l1a.tile([128, ws1], F32, tag="mma")
                                for kp in range(KA // 2):
                                    nc.tensor.matmul(
                                        pa[:, :],
                                        w1_t[:, o * KA + 2 * kp:
                                             o * KA + 2 * kp + 2, :],
                                        xt_t[0][:, 2 * kp:2 * kp + 2, :],
                                        start=(kp == 0),
                                        stop=(kp == KA // 2 - 1),
                                        perf_mode=DR,
                                    )
                                nc.scalar.activation(
                                    bsa[:, o, :], pa[:, :], ACTF.Identity,
                                    scale=ASCALE, bias=c1_t[:, o:o + 1])
                            for o in range(OC):
                                pb = l1b.tile([128, ws1], F32, tag="mmb")
                                for kp in range(KB // 2):
                                    nc.tensor.matmul(
                                        pb[:, :],
                                        w1_t[:, o * KA + 2 * kp:
                                             o * KA + 2 * kp + 2, :],
                                        xt_t[0][:, KA + 2 * kp:
                                                 KA + 2 * kp + 2, :],
                                        start=(kp == 0),
                                        stop=(kp == KB // 2 - 1),
                                        perf_mode=DR,
                                    )
                                # b1 = clip((B + bsb)*2^35) = sign(h1-mu1)
                                nc.vector._custom_dve(
                                    SIGNADD_OP, out=b1_t[:, o, s1],
                                    in0=pb[:, :], in1=bsa[:, o, :],
                                    s0=0.0, s1=float(2.0 ** 35))
                            continue

                        def l2_round(o2, n2, s2, ws2):
                            p2 = l23.tile([128, ws2], F32, tag="mm23",
                                          name="p2")
                            for kp in range(KH // 2):
                                nc.tensor.matmul(
                                    p2[:, :],
                                    w2_t[:, 2 * kp:2 * kp + 2,
                                         o2 * 128:(o2 + 1) * 128],
                                    b1_t[:, 2 * kp:2 * kp + 2, s2],
                                    start=(kp == 0),
                                    stop=(kp == KH // 2 - 1),
                                    perf_mode=DR,
                                )
                            nc.vector._custom_dve(
                                NOISY_OP, out=b2_t[:, o2, s2],
                                in0=p2[:, :], in1=a2_t[n2][:, o2, :],
                                s0=0.25, s1=32.0)

                        def l1_b_chain(o, pb):
                            """Group-B pairs accumulate onto the staged
                            PSUM bank (start=False), then b1 = Sign(PSUM)."""
                            for kp in range(KB // 2):
                                # start=False on an ACT-initialized bank is
                                # intentional: accumulate onto 2^15*A + c1.
                                nc.tensor.matmul(
                                    pb[:, :],
                                    w1_t[:, o * KA + 2 * kp:
                                         o * KA + 2 * kp + 2, :],
                                    xt_t[n1][:, KA + 2 * kp:
                                             KA + 2 * kp + 2, :],
                                    start=False,
                                    stop=(kp == KB // 2 - 1),
                                    perf_mode=DR,
                                    skip_group_check=True,
                                )
                            nc.scalar.activation(
                                b1_t[:, o, s1], pb[:, :], ACTF.Sign)

                        pb_prev = None  # (o, staged PSUM tile) from last round
                        for o in range(OC):
                            if n1 < NT:
                                pa = l1a.tile([128, ws1], F32, tag="mma")
                                for kp in range(KA // 2):
                                    nc.tensor.matmul(
                                        pa[:, :],
                                        w1_t[:, o * KA + 2 * kp:
                                             o * KA + 2 * kp + 2, :],
                                        xt_t[n1][:, 2 * kp:2 * kp + 2, :],
                                        start=(kp == 0),
                                        stop=(kp == KA // 2 - 1),
                                        perf_mode=DR,
                                    )
                                # PSUM init for group B: 2^15*A + c1
                                pb = l1b.tile([128, ws1], F32, tag="mmb")
                                nc.scalar.activation(
                                    pb[:, :], pa[:, :], ACTF.Identity,
                                    scale=ASCALE, bias=c1_t[:, o:o + 1])
                                if pb_prev is not None:
                                    l1_b_chain(*pb_prev)
                                pb_prev = (o, pb)
                            if o == 3 and l4_fin is not None:
                                l4_finish(l4_fin)
                                l4_fin = None
                            # Iteration 1's L2 rounds lag by 2 so the w2/a2
                            # DMAs (queued behind the front-load) land first.
                            o2 = o - 2 if i == 1 else o
                            if 0 <= n2 < NT - 1 and o2 >= 0:
                                l2_round(o2, n2, s2, ws2)
                            if 0 <= n3 < NT:
                                p3 = l23.tile([128, ws3], F32, tag="mm23",
                                              name="p3")
                                for kp in range(KH // 2):
                                    nc.tensor.matmul(
                                        p3[:, :],
                                        w3_t[:, 2 * kp:2 * kp + 2,
                                             o * 128:(o + 1) * 128],
                                        b2_t[:, 2 * kp:2 * kp + 2, s3],
                                        start=(kp == 0),
                                        stop=(kp == KH // 2 - 1),
                                        perf_mode=DR,
                                    )
                                nc.vector._custom_dve(
                                    NOISY_OP, out=b3_t[:, o, s3],
                                    in0=p3[:, :], in1=a3_t[n3][:, o, :],
                                    s0=0.25, s1=32.0)
                                if o >= 3 and o % 2 == 1:
                                    kp = (o - 3) // 2
                                    if kp == 0:
                                        ps4 = l4ps.tile([D_PAD4, ws3], F32,
                                                        tag="mm4")
                                        l4_pending = (ps4, n3)
                                    nc.tensor.matmul(
                                        ps4[:, :],
                                        w4_t[:, 2 * kp:2 * kp + 2, :],
                                        b3_t[:, 2 * kp:2 * kp + 2, s3],
                                        start=(kp == 0), stop=False,
                                        perf_mode=DR)
                        if pb_prev is not None:
                            l1_b_chain(*pb_prev)
                        if i == 1 and 0 <= n2 < NT - 1:
                            for o2 in (OC - 2, OC - 1):  # lagged catch-up
                                l2_round(o2, n2, s2, ws2)
                        if l4_fin is not None:  # iterations with empty loop
                            l4_finish(l4_fin)

                        # --- gamma-phase (last L1 iteration only): run the
                        # final slice's L2 immediately, so its noisy-sign DVE
                        # work overlaps the remaining tail PE work instead of
                        # serializing after it.
                        if i == NT - 1:
                            sg = s1
                            for o in range(OC):
                                p2 = l23.tile([128, ws1], F32, tag="mm23",
                                              name="p2g")
                                for kp in range(KH // 2):
                                    nc.tensor.matmul(
                                        p2[:, :],
                                        w2_t[:, 2 * kp:2 * kp + 2,
                                             o * 128:(o + 1) * 128],
                                        b1_t[:, 2 * kp:2 * kp + 2, sg],
                                        start=(kp == 0),
                                        stop=(kp == KH // 2 - 1),
                                        perf_mode=DR,
                                    )
                                nc.vector._custom_dve(
                                    NOISY_OP, out=b2_t[:, o, sg],
                                    in0=p2[:, :], in1=a2_t[n1][:, o, :],
                                    s0=0.25, s1=32.0)

    nc.compile()
    return nc


_NC_CACHE: dict[int, object] = {}


def _get_nc(repeat: int = 1):
    if repeat not in _NC_CACHE:
        _NC_CACHE[repeat] = build_nc(repeat)
    return _NC_CACHE[repeat]


def make_in_maps(x, u2, u3, W1, W2, W3, W4, **_unused):
    """Host preprocessing -> per-core input dicts."""
    fp8_np = mybir.dt.np(FP8)

    x = np.ascontiguousarray(np.asarray(x, dtype=np.float32))
    W1b = np.sign(np.asarray(W1, dtype=np.float32))
    # mean(h1, axis=0) = sign(W1) @ mean(x, axis=0), in float64; negated and
    # pre-scaled so the device computes sign(2^21 h1 + bias).
    mu1 = (W1b.astype(np.float64) @ x.mean(axis=0, dtype=np.float64)).astype(
        np.float32)
    c1 = np.ascontiguousarray(
        (np.float32(-XSCALE) * mu1).reshape(OC, 128).T)  # [128, OC]

    # balanced base-32 fixed-point split: round(x*2^21) = sum d_j 32^(4-j),
    # digits d_j in [-16, 16] - every one exact in fp8e4m3.
    assert np.abs(x).max() * XSCALE < 16.5 * (32 ** 5 - 1) / 31, "x overflow"
    n = np.rint(x.T.astype(np.float64) * XSCALE).astype(np.int64)  # [784, B]
    digs = []
    for _ in range(5):
        d = ((n + 16) % 32) - 16
        n = (n - d) >> 5
        digs.append(d)                      # LSB first: d4, d3, d2, d1, d0
    assert np.all(n == 0), "digit overflow"
    d4, d3, d2, d1, d0 = digs

    # xt slice layout [128, KXT, SW]: A stream (d0,d1,d2 = 2352 rows, pad
    # to KA*128) then B stream (d3,d4 = 1568 rows, pad to KB*128).
    xt_all = np.zeros((KXT * 128, B), dtype=fp8_np)
    for j, d in enumerate((d0, d1, d2)):
        xt_all[j * D_IN:(j + 1) * D_IN] = d.astype(np.float32).astype(fp8_np)
    for j, d in enumerate((d3, d4)):
        r = KA * 128 + j * D_IN
        xt_all[r:r + D_IN] = d.astype(np.float32).astype(fp8_np)

    # stored weight stream per out-chunk: [32*w, w, w/32] (2352 rows + pad),
    # shared by groups A and B (B reads the 2/3 prefix).
    w1p = np.zeros((KA * 128, D_H), dtype=fp8_np)
    for j, sc in enumerate((32.0, 1.0, 1.0 / 32.0)):
        w1p[j * D_IN:(j + 1) * D_IN] = (
            W1b.T * np.float32(sc)).astype(fp8_np)
    # [o][p][k][m]: one contiguous DMA per 128-feature output block.
    w1_blocks = np.ascontiguousarray(
        w1p.reshape(KA, 128, OC, 128).transpose(2, 1, 0, 3))

    pt = _prob_table()
    t2tab = _t2_table(fp8_np)
    a2f = t2tab[(_flip_thresholds(np.asarray(u2), pt) // 2)]   # [B, 1024]
    a3f = t2tab[(_flip_thresholds(np.asarray(u3), pt) // 2)]

    def _hidden_w(w, sc):
        wt = (np.sign(np.asarray(w, np.float32)) * np.float32(sc)
              ).T.astype(fp8_np)                               # [K, M]
        return np.ascontiguousarray(
            wt.reshape(KH, 128, wt.shape[1]).transpose(1, 0, 2))

    w2t = _hidden_w(W2, 0.5)               # [128, 8, 1024], +-0.5
    w3t = _hidden_w(W3, 0.5)
    w4t = _hidden_w(W4, 1.0)               # [128, 8, 10]
    w4p = np.zeros((128, KH, D_PAD4), dtype=fp8_np)
    w4p[:, :, :D_OUT] = w4t
    w4t = w4p

    in_maps = []
    for c in range(N_CORES):
        sl = slice(c * BC, (c + 1) * BC)
        m = {"w2": w2t, "w3": w3t, "w4": w4t, "c1": c1}
        xc = xt_all[:, sl].reshape(KXT, 128, BC)  # [k, p, col]
        for nn in range(NT):
            cs = slice(OFFS[nn], OFFS[nn] + SL[nn])
            m[f"xt{nn}"] = np.ascontiguousarray(
                xc[:, :, cs].transpose(1, 0, 2))
        for o in range(OC):
            m[f"w1_{o}"] = w1_blocks[o]
        for nm, tab in (("a2", a2f), ("a3", a3f)):
            rc = tab.T[:, sl].astype(fp8_np).reshape(OC, 128, BC)
            for nn in range(NT):
                cs = slice(OFFS[nn], OFFS[nn] + SL[nn])
                m[f"{nm}_{nn}"] = np.ascontiguousarray(
                    rc[:, :, cs].transpose(1, 0, 2))
        in_maps.append(m)
    return in_maps


def kernel(x, u2, u3, W1, W2, W3, W4,
           g1=None, b1=None, g2=None, b2=None, g3=None, b3=None):
    for g in (g1, g2, g3):
        assert g is None or np.all(np.asarray(g) > 0), "kernel assumes g > 0"
    for b in (b1, b2, b3):
        assert b is None or np.all(np.asarray(b) == 0), "kernel assumes b == 0"

    nc = _get_nc(repeat=1)
    in_maps = make_in_maps(x, u2, u3, W1, W2, W3, W4)
    res = run_bass_kernel_spmd(nc, in_maps, core_ids=list(range(N_CORES)))

    out = np.empty((B, D_OUT), dtype=np.float32)
    for c in range(N_CORES):
        out[c * BC:(c + 1) * BC, :] = res.results[c]["out"].T
    return out
